# revision 38
# baseline (speedup 1.0000x reference)
"""Trainium2 Bass kernel for the AdaptiveFF spiking network.

Sharding: data-parallel over batch, 8 NeuronCores, 32 batch elements per
core, weights replicated. No collectives needed.

Per-core kernel (all state feature-major: [feature_chunk=128, batch]):
  - X = inp @ W1.T is hoisted out of the sim loop (x_t is constant across
    the 4 sim steps) and batched per 8-step time block (fp32 matmuls).
  - The three recurrences (LIF v1, ALIF va/ba, LIF v2) are emitted as
    per-step interleaved DVE chains of three different time blocks
    (2-round software-pipeline skew, plus intra-round lags ALAG/VLAG), so
    the in-order engines always have independent work between the
    dependent ops of any one chain.
  - v1/v2 use a negated-state encoding u = s - w: the spike reset is a
    single fused STT (u = (w > 1) - w, exact fp32 parity with the
    reference) and the two chains' resets merge into one [128,256] op.
    Spike extraction for the matmul buffers runs off-chain on ScalarE as
    a saturated sigmoid (exact {0,1} outside ~1 ulp of threshold).
  - W2/W3 matmuls batch over (t, sim) with N=512 tiles and run as two
    passes: bf16 high + fp16 low residual, reproducing the fp32 product
    to ~1e-8 at 2x the fp32 rate. PSUM is evicted by ScalarE with the
    layer biases fused in.
  - The output integrator is folded into the W4 matmul by contracting
    over (sim_step, feature) against beta^(3-k)-scaled weight copies in a
    single fp16 pass (output-only path: fp16 weight rounding adds ~2e-4
    rel err with no feedback into the recurrence).
  - The ALIF chain's va/sa/ba/thr updates run on the otherwise-idle Pool
    (GpSimd) engine; only sth/va-reset stay on the DVE, balancing the
    three serial per-step chains across DVE+Pool+ACT under the PE's
    ~39us/block matmul floor.
Measured on trn2: ~1.28 ms HW exec at baseline, rel err ~0.015 vs the
fp32 numpy reference (the fp32 chaos floor of this net is ~0.013-0.014).
"""

import sys

for p in ("/opt/trn_rl_repo", "/root/.axon_site/_ro/trn_rl_repo"):
    if p not in sys.path:
        sys.path.append(p)

from contextlib import ExitStack

import numpy as np
import ml_dtypes

from concourse import mybir
import concourse.bass as bass
import concourse.tile as tile
from concourse.tile import TileContext
from concourse.bass_utils import run_bass_kernel_spmd

F32 = mybir.dt.float32
BF16 = mybir.dt.bfloat16
F16 = mybir.dt.float16
ALU = mybir.AluOpType
ACTF = mybir.ActivationFunctionType

T, B, NIN = 200, 256, 700
NS1, NA, NS2, NOUT = 512, 256, 512, 20
SIM = 4
BETA, THRESH, BETA_B, RHO = 0.9, 1.0, 0.95, 0.5
NCORES = 8
BC = B // NCORES          # 32 batch per core
TB = 8                    # time-block
NBLK = T // TB
NC1 = NS1 // 128          # 4 feature chunks for s1/x2
NC2 = NA // 128           # 2 chunks for sa/ax
NINP = NIN + 1            # 701: b1 rides the L1 matmul as a ones-row
NCI = (NINP + 127) // 128  # 6 input chunks (last ragged: 61)
NB = TB * BC              # 256 (t, b) cols per block
NKB = TB * SIM * BC       # 1024 (t, k, b) cols per block
NIT = 2 * NC1 + NC2       # 10 input-supertile chunks: X(x4 k-rep), x2, ax

_CACHE = {}


def _split_waits(nc, max_waits=1):
    """walrus in this container rejects >1 sem-wait per instruction; hoist
    extras onto preceding InstEventSemaphore instructions on the same
    engine (program order makes them happen-before)."""
    for f in nc.m.functions:
        for bb in f.blocks:
            dirty = False
            newl = []
            for ins in bb.instructions:
                si = ins.sync_info
                if si is not None and len(si.on_wait) > max_waits:
                    waits = list(si.on_wait)
                    for w in waits[:-max_waits]:
                        ev = mybir.InstEventSemaphore(
                            name=nc.get_next_instruction_name(), ins=[], outs=[])
                        ev.engine = ins.engine
                        ev.sync_info = mybir.SyncInfo(on_wait=[w], on_update=[])
                        nc.register_instruction(ev, overwrite=True)
                        newl.append(ev)
                    ins.sync_info = mybir.SyncInfo(
                        on_wait=waits[-max_waits:], on_update=list(si.on_update))
                    dirty = True
                newl.append(ins)
            if dirty:
                bb.instructions = newl


def _patch_tile_drain():
    if getattr(tile.TileContext, "_wait_split_patched", False):
        return
    orig = tile.TileContext._drain_and_barrier

    def patched(self, tick_clock, wait_clock):
        orig(self, tick_clock, wait_clock)
        _split_waits(self.nc)

    tile.TileContext._drain_and_barrier = patched
    tile.TileContext._wait_split_patched = True


def build_nc():
    _patch_tile_drain()
    nc = bass.Bass("TRN2", target_bir_lowering=False)

    dp = nc.declare_dram_parameter
    inpT = dp("inpT", [NINP, T, BC], F32, isOutput=False)
    w1t = dp("w1t", [NINP, NS1], F32, isOutput=False)
    w2h = dp("w2h", [NC1, NA // 128, 128, 128], BF16, isOutput=False)
    w2l = dp("w2l", [NC1, NA // 128, 128, 128], F16, isOutput=False)
    w3h = dp("w3h", [NC1 + NC2, NC1, 128, 128], BF16, isOutput=False)
    w3l = dp("w3l", [NC1 + NC2, NC1, 128, 128], F16, isOutput=False)
    w4f = dp("w4f", [SIM, NC1, 128, NOUT], F16, isOutput=False)
    b2m = dp("b2m", [NC2, 128], F32, isOutput=False)
    b3m = dp("b3m", [NC1, 128], F32, isOutput=False)
    b4c = dp("b4c", [NOUT, 1], F32, isOutput=False)
    outT = dp("outT", [NOUT, T, BC], F32, isOutput=True)

    with TileContext(nc) as tc, ExitStack() as ctx:
        wpool = ctx.enter_context(tc.tile_pool(name="weights", bufs=1))
        spool = ctx.enter_context(tc.tile_pool(name="states", bufs=1))
        szpool = ctx.enter_context(tc.tile_pool(name="szbuf", bufs=3))
        sapool = ctx.enter_context(tc.tile_pool(name="sabuf", bufs=3))
        itpool = ctx.enter_context(tc.tile_pool(name="itbuf", bufs=2))
        ipool = ctx.enter_context(tc.tile_pool(name="inp", bufs=2))
        opool = ctx.enter_context(tc.tile_pool(name="outt", bufs=2))
        pxpool = ctx.enter_context(tc.tile_pool(name="px", bufs=2, space="PSUM"))
        pmpool = ctx.enter_context(tc.tile_pool(name="pmid", bufs=5, space="PSUM"))
        popool = ctx.enter_context(tc.tile_pool(name="po", bufs=1, space="PSUM"))

        # ---- load weights ----
        w1 = []
        for c in range(NCI):
            kc = min(128, NINP - c * 128)
            wt = wpool.tile([kc, NS1], F32, tag=f"w1_{c}", name=f"w1_{c}")
            nc.sync.dma_start(out=wt[:], in_=w1t[c * 128:c * 128 + kc, :])
            w1.append(wt)
        def load_blocks(dram, nctot, nm, dt_, nm_name):
            # one DMA per K-chunk: [128, nm*128] tile whose m-th 128-col
            # slice is the contiguous [128,128] block (c, m)
            tiles = []
            for c in range(nctot):
                wt = wpool.tile([128, nm * 128], dt_, tag=f"{nm_name}_{c}",
                                name=f"{nm_name}_{c}")
                nc.sync.dma_start(
                    out=wt[:],
                    in_=bass.AP(dram, c * nm * 128 * 128,
                                [[128, 128], [128 * 128, nm], [1, 128]]))
                tiles.append([wt[:, m * 128:(m + 1) * 128] for m in range(nm)])
            return tiles

        w2hp, w3hp, w2lp, w3lp = [], [], [], []
        HEAVY = []
        HEAVY.append(lambda: w2hp.extend(load_blocks(w2h, NC1, NA // 128, BF16, "w2h")))
        HEAVY.append(lambda: w3hp.extend(load_blocks(w3h, NC1 + NC2, NC1, BF16, "w3h")))
        HEAVY.append(lambda: w2lp.extend(load_blocks(w2l, NC1, NA // 128, F16, "w2l")))
        HEAVY.append(lambda: w3lp.extend(load_blocks(w3l, NC1 + NC2, NC1, F16, "w3l")))
        w4fp = []

        def _load_w4():
            for k in range(SIM):
                rf = []
                for c in range(NC1):
                    wt = wpool.tile([128, NOUT], F16, tag=f"w4f_{k}_{c}",
                                    name=f"w4f_{k}_{c}")
                    nc.sync.dma_start(out=wt[:], in_=w4f[k, c, :, :])
                    rf.append(wt)
                w4fp.append(rf)
        HEAVY.append(_load_w4)
        b2t = wpool.tile([128, NC2], F32, tag="b2t", name="b2t")
        nc.sync.dma_start(out=b2t[:], in_=bass.AP(b2m, 0, [[1, 128], [128, NC2]]))
        b3t = wpool.tile([128, NC1], F32, tag="b3t", name="b3t")
        nc.sync.dma_start(out=b3t[:], in_=bass.AP(b3m, 0, [[1, 128], [128, NC1]]))
        b4t = wpool.tile([NOUT, 1], F32, tag="b4t", name="b4t")
        nc.sync.dma_start(out=b4t[:], in_=b4c[:, :])

        # ---- persistent states, layout [128, chunk*BC + b] ----
        # All three chains use the negated-state encoding u = s*th - w, so
        # the leak+input update is w' = (u * -BETA) + x for every chain and
        # the three updates merge into ONE 320-col STT (the ALIF state is
        # stored as nva = sth - wa = -va, exact in fp32).
        # ucat2/wcat2 layout: [u1 (128c) | u2 (128c) | nva (64c)].
        SW = NC1 * BC
        AW = NC2 * BC
        TW = 2 * SW + AW
        wcat2 = [spool.tile([128, TW], F32, tag=f"wcat{j}", name=f"wcat{j}")
                 for j in range(2)]
        ucat2 = spool.tile([128, TW], F32, tag="ucat", name="ucat")
        ba = spool.tile([128, AW], F32, tag="ba", name="ba")
        thr = spool.tile([128, AW], F32, tag="thr", name="thr")
        sth = spool.tile([128, AW], F32, tag="sth", name="sth")
        nc.vector.memset(ba[:], 0.0)
        nc.vector.memset(ucat2[:], 0.0)

        # sigmoid-spike bias tile: s = sigmoid(SIGS*(v - THRESH)) saturates
        # to exact {0,1} outside ~1 ulp of the threshold
        SIGS = 1e8
        bsig = wpool.tile([128, 1], F32, tag="bsig", name="bsig")
        nc.vector.memset(bsig[:], -SIGS * THRESH)

        # per-block tiles carried between skewed emission rounds
        S1 = [None] * NBLK
        SA = [None] * NBLK
        # ITS[i]: input supertile read by chains(i): chunks 0..3 = X(i)
        # k-replicated, 4..7 = x2(i-2), 8..9 = ax(i-1); all three writers
        # (L1(i) evict, L3(i-2) evict, L2(i-1) evict) run in round i-1.
        ITS = [None] * (NBLK + 2)

        def emit_inp_l1(i):
            """inp DMA, L1 matmuls (bias rides as the 701st ones-row), X
            eviction k-replicated x4 into ITS[i] chunks 0..3."""
            t0 = i * TB
            itiles = []
            for c in range(NCI):
                kc = min(128, NINP - c * 128)
                it = ipool.tile([kc, NB], F32, tag=f"inp_{c}", name=f"inp_{c}")
                nc.sync.dma_start(
                    out=it[:],
                    in_=bass.AP(inpT, c * 128 * T * BC + t0 * BC,
                                [[T * BC, kc], [1, NB]]))
                itiles.append(it)
            px = [pxpool.tile([128, 2 * NB], F32, tag="px", name="px") for _ in range(2)]
            for mt in range(2):
                for m2 in range(2):
                    m = 2 * mt + m2
                    for c in range(NCI):
                        nc.tensor.matmul(
                            px[mt][:, m2 * NB:(m2 + 1) * NB],
                            w1[c][:, m * 128:(m + 1) * 128],
                            itiles[c][:],
                            start=(c == 0), stop=(c == NCI - 1))
            itr = ITS[i].rearrange("p (c t k b) -> p c t k b", c=NIT, t=TB,
                                   k=SIM)
            for mt in range(2):
                pxr = px[mt].rearrange("p (m t b) -> p m t b", m=2, t=TB)
                for k in range(SIM):
                    nc.scalar.activation(
                        itr[:, 2 * mt:2 * mt + 2, :, k, :], pxr[:],
                        ACTF.Identity)

        def emit_chains(i):
            """Interleaved per-step emission of the three state chains at
            the SAME step index: v1/s1 of block i, ALIF of block i-1,
            v2/s2/z of block i-2. All three leak+input updates merge into
            one 320-col STT (reading the ITS[i] input supertile); both
            spike extractions merge into one ACT op; the ALIF threshold
            ops stay separate (thr is a tensor)."""
            NS = TB * SIM
            sz = sar = None
            hasV1 = 0 <= i < NBLK
            hasAl = 0 <= i - 1 < NBLK
            hasV2 = 0 <= i - 2 < NBLK
            if hasV1 or hasV2:
                # spike supertile: chunks 0..NC1-1 hold s1 of block i,
                # chunks NC1..2*NC1-1 hold s2 (zb) of block i-2
                sz = szpool.tile([128, 2 * NC1 * NKB], BF16, tag="SZ",
                                 name="SZ")
                szr = sz.rearrange("p (c t k b) -> p c t k b", c=2 * NC1,
                                   t=TB, k=SIM)
            if hasV1:
                S1[i] = sz
            if hasAl:
                saT = sapool.tile([128, NC2 * NKB], BF16, tag="SA", name="SA")
                SA[i - 1] = saT
                sar = saT.rearrange("p (c t k b) -> p c t k b", c=NC2, t=TB,
                                    k=SIM)
                nc.scalar.activation(
                    thr[:], ba[:], ACTF.Identity, bias=THRESH, scale=RHO)
            ITr = ITS[i].rearrange("p (c t k b) -> p c t k b", c=NIT, t=TB,
                                   k=SIM)

            # coalesced contiguous (state_lo, state_hi, chunk_lo, chunk_hi)
            # segments for the merged leak+input STT
            wsegs = []
            for act, lo, hi, clo, chi in (
                    (hasV1, 0, SW, 0, NC1),
                    (hasV2, SW, 2 * SW, NC1, 2 * NC1),
                    (hasAl, 2 * SW, TW, 2 * NC1, NIT)):
                if not act:
                    continue
                if wsegs and wsegs[-1][1] == lo:
                    wsegs[-1][1] = hi
                    wsegs[-1][3] = chi
                else:
                    wsegs.append([lo, hi, clo, chi])

            def merged_w(s):
                tt, k = divmod(s, SIM)
                for lo, hi, clo, chi in wsegs:
                    nc.vector.scalar_tensor_tensor(
                        wcat2[s % 2][:, lo:hi], ucat2[:, lo:hi], -BETA,
                        ITr[:, clo:chi, tt, k, :], ALU.mult, ALU.add)

            def u_reset(s):
                w = wcat2[s % 2]
                if hasV1 and hasV2:
                    nc.vector.scalar_tensor_tensor(
                        ucat2[:, 0:2 * SW], w[:, 0:2 * SW], THRESH,
                        w[:, 0:2 * SW], ALU.is_gt, ALU.subtract)
                elif hasV1:
                    nc.vector.scalar_tensor_tensor(
                        ucat2[:, 0:SW], w[:, 0:SW], THRESH, w[:, 0:SW],
                        ALU.is_gt, ALU.subtract)
                elif hasV2:
                    nc.vector.scalar_tensor_tensor(
                        ucat2[:, SW:2 * SW], w[:, SW:2 * SW], THRESH,
                        w[:, SW:2 * SW], ALU.is_gt, ALU.subtract)

            def sig(s):
                """merged spike extraction for v1 (s1, chunks 0..NC1-1) and
                v2 (zb, chunks NC1..2*NC1-1) at the same step index"""
                tt, k = divmod(s, SIM)
                if hasV1 and hasV2:
                    nc.scalar.activation(
                        szr[:, :, tt, k, :],
                        wcat2[s % 2][:, 0:2 * SW].rearrange(
                            "p (c b) -> p c b", c=2 * NC1),
                        ACTF.Sigmoid, bias=bsig[:, 0:1], scale=SIGS)
                elif hasV1:
                    nc.scalar.activation(
                        szr[:, 0:NC1, tt, k, :],
                        wcat2[s % 2][:, 0:SW].rearrange("p (c b) -> p c b",
                                                        c=NC1),
                        ACTF.Sigmoid, bias=bsig[:, 0:1], scale=SIGS)
                elif hasV2:
                    nc.scalar.activation(
                        szr[:, NC1:2 * NC1, tt, k, :],
                        wcat2[s % 2][:, SW:2 * SW].rearrange(
                            "p (c b) -> p c b", c=NC1),
                        ACTF.Sigmoid, bias=bsig[:, 0:1], scale=SIGS)

            nva = ucat2[:, 2 * SW:TW]

            def alif_ops(s):
                """threshold compare, soft reset (as nva = sth - wa, exact
                -va), adaptation, and next-step thr for alif step s"""
                tt, k = divmod(s, SIM)
                saap = sar[:, :, tt, k, :]
                wa = wcat2[s % 2][:, 2 * SW:TW]
                nc.vector.tensor_tensor(saap, wa, thr[:], ALU.is_gt)
                nc.vector.tensor_tensor(sth[:], saap, thr[:], ALU.mult)
                nc.vector.tensor_tensor(nva, sth[:], wa, ALU.subtract)
                nc.vector.scalar_tensor_tensor(
                    ba[:], ba[:], BETA_B, saap, ALU.mult, ALU.add)
                nc.scalar.activation(
                    thr[:], ba[:], ACTF.Identity, bias=THRESH, scale=RHO)

            for s in range(NS):
                merged_w(s)
                if hasV1 or hasV2:
                    u_reset(s)
                    sig(s)
                if hasAl:
                    alif_ops(s)
            return sz

        def emit_l2(i):
            # ax(i) lands in ITS[i+1] chunks 2*NC1..2*NC1+NC2-1
            s1 = S1[i]
            it = ITS[i + 1]
            for m2 in range(NC2):
                pms = [pmpool.tile([128, 512], F32, tag="pm", name="pm")
                       for _ in range(NKB // 512)]
                for ph, wp in enumerate((w2hp, w2lp)):
                    for c in range(NC1):
                        for n in range(NKB // 512):
                            nc.tensor.matmul(
                                pms[n][:],
                                wp[c][m2][:],
                                s1[:, c * NKB + n * 512:c * NKB + (n + 1) * 512],
                                start=(ph == 0 and c == 0),
                                stop=(ph == 1 and c == NC1 - 1))
                base = (2 * NC1 + m2) * NKB
                for n in range(NKB // 512):
                    nc.scalar.activation(
                        it[:, base + n * 512:base + (n + 1) * 512],
                        pms[n][:], ACTF.Identity, bias=b2t[:, m2:m2 + 1])

        def emit_l3(i):
            # x2(i) lands in ITS[i+2] chunks NC1..2*NC1-1
            s1 = S1[i]
            sa = SA[i]
            it = ITS[i + 2]
            for m in range(NC1):
                pms = [pmpool.tile([128, 512], F32, tag="pm", name="pm")
                       for _ in range(NKB // 512)]
                for ph, wp in enumerate((w3hp, w3lp)):
                    for c in range(NC1):
                        for n in range(NKB // 512):
                            nc.tensor.matmul(
                                pms[n][:],
                                wp[c][m][:],
                                s1[:, c * NKB + n * 512:c * NKB + (n + 1) * 512],
                                start=(ph == 0 and c == 0), stop=False)
                    for c2 in range(NC2):
                        for n in range(NKB // 512):
                            nc.tensor.matmul(
                                pms[n][:],
                                wp[NC1 + c2][m][:],
                                sa[:, c2 * NKB + n * 512:c2 * NKB + (n + 1) * 512],
                                start=False,
                                stop=(ph == 1 and c2 == NC2 - 1))
                base = (NC1 + m) * NKB
                for n in range(NKB // 512):
                    nc.scalar.activation(
                        it[:, base + n * 512:base + (n + 1) * 512],
                        pms[n][:], ACTF.Identity, bias=b3t[:, m:m + 1])

        def emit_l4(i, sz):
            t0 = i * TB
            zbr = sz.rearrange("p (c t k b) -> p c t k b", c=2 * NC1, t=TB,
                               k=SIM)
            po = popool.tile([NOUT, NB], F32, tag="po", name="po")
            first = True
            for k in range(SIM):
                for c in range(NC1):
                    nc.tensor.matmul(
                        po[:], w4fp[k][c][:], zbr[:, NC1 + c, :, k, :],
                        start=first,
                        stop=(k == SIM - 1 and c == NC1 - 1))
                    first = False
            ot = opool.tile([NOUT, NB], F32, tag="OT", name="OT")
            nc.scalar.activation(ot[:], po[:], ACTF.Identity, bias=b4t[:, 0:1])
            nc.sync.dma_start(
                out=bass.AP(outT, t0 * BC, [[T * BC, NOUT], [1, NB]]),
                in_=ot[:])

        # software-pipelined emission with 2-round skew; L1 of the NEXT
        # block leads each round so PE has boundary work.  ITS[r+1] is
        # allocated one round ahead: all its writers (L1(r+1), L2(r),
        # L3(r-1) evictions) run during round r.
        def alloc_it(j):
            if 0 <= j < NBLK + 2 and ITS[j] is None:
                ITS[j] = itpool.tile([128, NIT * NKB], F32, tag="IT",
                                     name="IT")
        alloc_it(0)
        emit_inp_l1(0)
        for fn in HEAVY:
            fn()
        for r in range(NBLK + 2):
            alloc_it(r + 1)
            if r + 1 < NBLK:
                emit_inp_l1(r + 1)
            zb = emit_chains(r)
            if 1 <= r < NBLK + 1:
                emit_l3(r - 1)
            if r < NBLK:
                emit_l2(r)
            if r >= 2:
                emit_l4(r - 2, zb)

    return nc


def _prep_host(inputs):
    inp = np.ascontiguousarray(inputs["inp"], dtype=np.float32)
    W1 = np.asarray(inputs["W1"], np.float32)
    W2 = np.asarray(inputs["W2"], np.float32)
    W3 = np.asarray(inputs["W3"], np.float32)
    W4 = np.asarray(inputs["W4"], np.float32)
    b1 = np.asarray(inputs["b1"], np.float32)
    b2 = np.asarray(inputs["b2"], np.float32)
    b3 = np.asarray(inputs["b3"], np.float32)
    b4 = np.asarray(inputs["b4"], np.float32)

    def split(W):
        # [K, M] -> [K//128, M//128, 128, 128] contiguous blocks (FWL needs
        # contiguous weight tiles)
        WT = W.T
        K, M = WT.shape
        Wh = WT.astype(ml_dtypes.bfloat16)
        Wl = (WT - Wh.astype(np.float32)).astype(np.float16)
        def blk(A):
            return np.ascontiguousarray(
                A.reshape(K // 128, 128, M // 128, 128).transpose(0, 2, 1, 3))
        return blk(Wh), blk(Wl)

    w2h, w2l = split(W2)
    w3h, w3l = split(W3)
    # W4cat: per sim step k the output integrator weight is beta^(SIM-1-k)*W4
    W4T = W4.T.astype(np.float64)                     # [NS2, NOUT]
    w4cat = np.stack([(BETA ** (SIM - 1 - k)) * W4T for k in range(SIM)])
    w4cat = w4cat.reshape(SIM, NC1, 128, NOUT)
    w4f_ = w4cat.astype(np.float16)
    csum = float(sum(BETA ** k for k in range(SIM)))
    # b1 rides the L1 matmul as weight row NIN (ones appended to inpT)
    w1cat = np.vstack([W1.T, b1[None, :]])                     # [NIN+1, NS1]
    shared = dict(
        w1t=np.ascontiguousarray(w1cat),
        w2h=w2h, w2l=w2l, w3h=w3h, w3l=w3l,
        w4f=np.ascontiguousarray(w4f_),
        b2m=np.ascontiguousarray(b2.reshape(NC2, 128)),
        b3m=np.ascontiguousarray(b3.reshape(NC1, 128)),
        b4c=np.ascontiguousarray((b4.astype(np.float64) * csum)
                                 .astype(np.float32).reshape(NOUT, 1)),
    )
    ones_row = np.ones((1, T, BC), np.float32)
    in_maps = []
    for c in range(NCORES):
        shard = inp[:, c * BC:(c + 1) * BC, :]                 # [T, BC, NIN]
        m = dict(shared)
        m["inpT"] = np.ascontiguousarray(
            np.concatenate([shard.transpose(2, 0, 1), ones_row], axis=0))
        in_maps.append(m)
    return in_maps


def run(inputs, trace=False, **kw):
    if "nc" not in _CACHE:
        _CACHE["nc"] = build_nc()
    nc = _CACHE["nc"]
    in_maps = _prep_host(inputs)
    res = run_bass_kernel_spmd(nc, in_maps, core_ids=list(range(NCORES)),
                               trace=trace, **kw)
    outs = []
    for c in range(NCORES):
        outT = res.results[c]["outT"]                          # [NOUT, T, BC]
        outs.append(np.ascontiguousarray(outT.transpose(1, 2, 0)))
    full = np.concatenate(outs, axis=1)                        # [T, B, NOUT]
    return full, res


def kernel(**inputs):
    out, _ = run(inputs)
    return out



# revision 43
# speedup vs baseline: 1.0834x; 1.0834x over previous
"""Trainium2 Bass kernel for the AdaptiveFF spiking network.

Sharding: data-parallel over batch, 8 NeuronCores, 32 batch elements per
core, weights replicated. No collectives needed.

Per-core kernel (all state feature-major: [feature_chunk=128, batch]):
  - X = inp @ W1.T is hoisted out of the sim loop (x_t is constant across
    the 4 sim steps) and batched per 8-step time block; b1 rides the
    matmul as a 701st ones-row, so the PSUM eviction is a bias-free copy
    that runs on the otherwise-idle GpSimd engine.
  - The three recurrences (LIF v1, ALIF va/ba, LIF v2) are emitted as
    per-step interleaved chains; ALIF lags 8 steps so the in-order
    engines always have independent work between the dependent ops of
    any one chain.
  - v1/v2 use a negated-state encoding u = s - w: the spike reset is a
    single fused STT (u = (w > 1) - w, exact fp32 parity with the
    reference) and the two chains' resets merge into one [128,256] op.
    v1 and v2 run at the same step index and their spike extractions
    merge into ONE saturated-sigmoid ACT op per step writing a shared
    s1|zb supertile (exact {0,1} outside ~1 ulp of threshold).
  - ALIF per step: w = beta*va+ax (STT), d = w - thr (TT), spike via
    saturated sigmoid of d on ACT (writes the bf16 SA matmul buffer),
    soft reset via copy_predicated(va, sa, d) (bit-exact: d = w - thr
    equals the reference's w - sa*thr when sa=1), ba STT, thr on ACT.
    The d->sa->reset round trip is software-pipelined one outer step
    deep (front: w,d; back: reset,ba) to hide the ACT latency.
  - W2/W3 matmuls batch over (t, sim) with N=512 tiles and run as two
    passes: bf16 high + fp16 low residual, reproducing the fp32 product
    to ~1e-8 at 2x the fp32 rate. PSUM is evicted by ScalarE with the
    layer biases fused in.
  - The output integrator is folded into the W4 matmul by contracting
    over (sim_step, feature) against beta^(3-k)-scaled weight copies in
    a single fp16 pass (output-only path, no feedback).
Measured on trn2: baseline 1.283 ms; rel err ~0.015 vs the fp32 numpy
reference (the fp32 chaos floor of this spiking net is ~0.013-0.014).
"""

import sys

for p in ("/opt/trn_rl_repo", "/root/.axon_site/_ro/trn_rl_repo"):
    if p not in sys.path:
        sys.path.append(p)

from contextlib import ExitStack

import numpy as np
import ml_dtypes

from concourse import mybir
import concourse.bass as bass
import concourse.tile as tile
from concourse.tile import TileContext
from concourse.bass_utils import run_bass_kernel_spmd

F32 = mybir.dt.float32
BF16 = mybir.dt.bfloat16
F16 = mybir.dt.float16
ALU = mybir.AluOpType
ACTF = mybir.ActivationFunctionType

T, B, NIN = 200, 256, 700
NS1, NA, NS2, NOUT = 512, 256, 512, 20
SIM = 4
BETA, THRESH, BETA_B, RHO = 0.9, 1.0, 0.95, 0.5
NCORES = 8
BC = B // NCORES          # 32 batch per core
TB = 8                    # time-block
NBLK = T // TB
NC1 = NS1 // 128          # 4 feature chunks for s1/x2
NC2 = NA // 128           # 2 chunks for sa/ax
NINP = NIN + 1            # 701: b1 rides the L1 matmul as a ones-row
NCI = (NINP + 127) // 128  # 6 input chunks (last ragged: 61)
NB = TB * BC              # 256 (t, b) cols per block
NKB = TB * SIM * BC       # 1024 (t, k, b) cols per block

_CACHE = {}


def _split_waits(nc, max_waits=1):
    """walrus in this container rejects >1 sem-wait per instruction; hoist
    extras onto preceding InstEventSemaphore instructions on the same
    engine (program order makes them happen-before)."""
    for f in nc.m.functions:
        for bb in f.blocks:
            dirty = False
            newl = []
            for ins in bb.instructions:
                si = ins.sync_info
                if si is not None and len(si.on_wait) > max_waits:
                    waits = list(si.on_wait)
                    for w in waits[:-max_waits]:
                        ev = mybir.InstEventSemaphore(
                            name=nc.get_next_instruction_name(), ins=[], outs=[])
                        ev.engine = ins.engine
                        ev.sync_info = mybir.SyncInfo(on_wait=[w], on_update=[])
                        nc.register_instruction(ev, overwrite=True)
                        newl.append(ev)
                    ins.sync_info = mybir.SyncInfo(
                        on_wait=waits[-max_waits:], on_update=list(si.on_update))
                    dirty = True
                newl.append(ins)
            if dirty:
                bb.instructions = newl


def _patch_tile_drain():
    if getattr(tile.TileContext, "_wait_split_patched", False):
        return
    orig = tile.TileContext._drain_and_barrier

    def patched(self, tick_clock, wait_clock):
        orig(self, tick_clock, wait_clock)
        _split_waits(self.nc)

    tile.TileContext._drain_and_barrier = patched
    tile.TileContext._wait_split_patched = True


def build_nc():
    _patch_tile_drain()
    nc = bass.Bass("TRN2", target_bir_lowering=False)

    dp = nc.declare_dram_parameter
    inpT = dp("inpT", [NINP, T, BC], F32, isOutput=False)
    w1t = dp("w1t", [NINP, NS1], F32, isOutput=False)
    w2h = dp("w2h", [NC1, NA // 128, 128, 128], BF16, isOutput=False)
    w2l = dp("w2l", [NC1, NA // 128, 128, 128], F16, isOutput=False)
    w3h = dp("w3h", [NC1 + NC2, NC1, 128, 128], BF16, isOutput=False)
    w3l = dp("w3l", [NC1 + NC2, NC1, 128, 128], F16, isOutput=False)
    w4f = dp("w4f", [SIM, NC1, 128, NOUT], F16, isOutput=False)
    b2m = dp("b2m", [NC2, 128], F32, isOutput=False)
    b3m = dp("b3m", [NC1, 128], F32, isOutput=False)
    b4c = dp("b4c", [NOUT, 1], F32, isOutput=False)
    outT = dp("outT", [NOUT, T, BC], F32, isOutput=True)

    with TileContext(nc) as tc, ExitStack() as ctx:
        wpool = ctx.enter_context(tc.tile_pool(name="weights", bufs=1))
        spool = ctx.enter_context(tc.tile_pool(name="states", bufs=1))
        xpool = ctx.enter_context(tc.tile_pool(name="xbuf", bufs=3))
        szpool = ctx.enter_context(tc.tile_pool(name="szbuf", bufs=3))
        sapool = ctx.enter_context(tc.tile_pool(name="sabuf", bufs=3))
        axpool = ctx.enter_context(tc.tile_pool(name="axbuf", bufs=3))
        x2pool = ctx.enter_context(tc.tile_pool(name="x2buf", bufs=3))
        ipool = ctx.enter_context(tc.tile_pool(name="inp", bufs=2))
        opool = ctx.enter_context(tc.tile_pool(name="outt", bufs=2))
        pxpool = ctx.enter_context(tc.tile_pool(name="px", bufs=2, space="PSUM"))
        pmpool = ctx.enter_context(tc.tile_pool(name="pmid", bufs=5, space="PSUM"))
        popool = ctx.enter_context(tc.tile_pool(name="po", bufs=1, space="PSUM"))

        # ---- load weights ----
        w1 = []
        for c in range(NCI):
            kc = min(128, NINP - c * 128)
            wt = wpool.tile([kc, NS1], F32, tag=f"w1_{c}", name=f"w1_{c}")
            nc.sync.dma_start(out=wt[:], in_=w1t[c * 128:c * 128 + kc, :])
            w1.append(wt)

        def load_blocks(dram, nctot, nm, dt_, nm_name):
            # one DMA per K-chunk: [128, nm*128] tile whose m-th 128-col
            # slice is the contiguous [128,128] block (c, m)
            tiles = []
            for c in range(nctot):
                wt = wpool.tile([128, nm * 128], dt_, tag=f"{nm_name}_{c}",
                                name=f"{nm_name}_{c}")
                nc.sync.dma_start(
                    out=wt[:],
                    in_=bass.AP(dram, c * nm * 128 * 128,
                                [[128, 128], [128 * 128, nm], [1, 128]]))
                tiles.append([wt[:, m * 128:(m + 1) * 128] for m in range(nm)])
            return tiles

        w2hp, w3hp, w2lp, w3lp = [], [], [], []
        HEAVY = []
        HEAVY.append(lambda: w2hp.extend(load_blocks(w2h, NC1, NA // 128, BF16, "w2h")))
        HEAVY.append(lambda: w3hp.extend(load_blocks(w3h, NC1 + NC2, NC1, BF16, "w3h")))
        HEAVY.append(lambda: w2lp.extend(load_blocks(w2l, NC1, NA // 128, F16, "w2l")))
        HEAVY.append(lambda: w3lp.extend(load_blocks(w3l, NC1 + NC2, NC1, F16, "w3l")))
        w4fp = []

        def _load_w4():
            for k in range(SIM):
                rf = []
                for c in range(NC1):
                    wt = wpool.tile([128, NOUT], F16, tag=f"w4f_{k}_{c}",
                                    name=f"w4f_{k}_{c}")
                    nc.sync.dma_start(out=wt[:], in_=w4f[k, c, :, :])
                    rf.append(wt)
                w4fp.append(rf)
        HEAVY.append(_load_w4)
        b2t = wpool.tile([128, NC2], F32, tag="b2t", name="b2t")
        nc.sync.dma_start(out=b2t[:], in_=bass.AP(b2m, 0, [[1, 128], [128, NC2]]))
        b3t = wpool.tile([128, NC1], F32, tag="b3t", name="b3t")
        nc.sync.dma_start(out=b3t[:], in_=bass.AP(b3m, 0, [[1, 128], [128, NC1]]))
        b4t = wpool.tile([NOUT, 1], F32, tag="b4t", name="b4t")
        nc.sync.dma_start(out=b4t[:], in_=b4c[:, :])

        # ---- persistent states, layout [128, chunk*BC + b] ----
        SW = NC1 * BC
        AW = NC2 * BC
        wcat = [spool.tile([128, 2 * SW], F32, tag=f"wcat{j}", name=f"wcat{j}")
                for j in range(2)]
        ucat = spool.tile([128, 2 * SW], F32, tag="ucat", name="ucat")
        u1s = ucat[:, 0:SW]
        u2s = ucat[:, SW:2 * SW]
        va = spool.tile([128, AW], F32, tag="va", name="va")
        ba = spool.tile([128, AW], F32, tag="ba", name="ba")
        thr = spool.tile([128, AW], F32, tag="thr", name="thr")
        dtl = spool.tile([128, AW], F32, tag="dtl", name="dtl")
        for st in (va, ba):
            nc.vector.memset(st[:], 0.0)
        nc.vector.memset(ucat[:], 0.0)

        # sigmoid-spike bias tile: s = sigmoid(SIGS*(v - THRESH)) saturates
        # to exact {0,1} outside ~1 ulp of the threshold
        SIGS = 1e8
        bsig = wpool.tile([128, 1], F32, tag="bsig", name="bsig")
        nc.vector.memset(bsig[:], -SIGS * THRESH)

        # per-block tiles carried between skewed emission rounds
        S1 = [None] * NBLK
        SA = [None] * NBLK
        AXB = [None] * NBLK
        X2B = [None] * NBLK
        XS = [None] * NBLK

        def emit_inp_l1(i):
            """inp DMA, L1 matmuls (b1 rides as the 701st ones-row), X
            eviction as two bias-free [128,512] ACT copies."""
            t0 = i * TB
            itiles = []
            for c in range(NCI):
                kc = min(128, NINP - c * 128)
                it = ipool.tile([kc, NB], F32, tag=f"inp_{c}", name=f"inp_{c}")
                nc.sync.dma_start(
                    out=it[:],
                    in_=bass.AP(inpT, c * 128 * T * BC + t0 * BC,
                                [[T * BC, kc], [1, NB]]))
                itiles.append(it)
            px = [pxpool.tile([128, 2 * NB], F32, tag="px", name="px")
                  for _ in range(2)]
            for mt in range(2):
                for m2 in range(2):
                    m = 2 * mt + m2
                    for c in range(NCI):
                        nc.tensor.matmul(
                            px[mt][:, m2 * NB:(m2 + 1) * NB],
                            w1[c][:, m * 128:(m + 1) * 128],
                            itiles[c][:],
                            start=(c == 0), stop=(c == NCI - 1))
            X = xpool.tile([128, NC1 * NB], F32, tag="X", name="X")
            XS[i] = X
            for mt in range(2):
                nc.scalar.activation(
                    X[:, mt * 2 * NB:(mt + 1) * 2 * NB], px[mt][:],
                    ACTF.Identity)

        def emit_chains(i):
            """Interleaved per-step emission of the three state chains:
            v1/s1 of block i, ALIF of block i-1 (lagged 8 steps so the
            L2(i-1) psum evictions land first), v2/s2/z of block i-2 (same
            step index as v1, so both spike extractions merge into one ACT
            op). Interleaving keeps the in-order DVE busy during the ACT
            round-trips of each individual chain."""
            NS = TB * SIM
            ALAG = 8
            s1 = saT = x2 = sz = None
            Xr = szr = axr = sar = x2r = None
            if 0 <= i < NBLK or 0 <= i - 2 < NBLK:
                # spike supertile: chunks 0..NC1-1 hold s1 of block i,
                # chunks NC1..2*NC1-1 hold s2 (zb) of block i-2
                sz = szpool.tile([128, 2 * NC1 * NKB], BF16, tag="SZ",
                                 name="SZ")
                szr = sz.rearrange("p (c t k b) -> p c t k b", c=2 * NC1,
                                   t=TB, k=SIM)
            if 0 <= i < NBLK:
                s1 = sz
                S1[i] = sz
                Xr = XS[i].rearrange("p (m t b) -> p m t b", m=NC1, t=TB)
            if 0 <= i - 1 < NBLK:
                ax = AXB[i - 1]
                saT = sapool.tile([128, NC2 * NKB], BF16, tag="SA", name="SA")
                SA[i - 1] = saT
                axr = ax.rearrange("p (c t k b) -> p c t k b", c=NC2, t=TB, k=SIM)
                sar = saT.rearrange("p (c t k b) -> p c t k b", c=NC2, t=TB, k=SIM)
                nc.scalar.activation(
                    thr[:], ba[:], ACTF.Identity, bias=THRESH, scale=RHO)

            if 0 <= i - 2 < NBLK:
                x2 = X2B[i - 2]
                x2r = x2.rearrange("p (c t k b) -> p c t k b", c=NC1, t=TB, k=SIM)

            def v1_step(s):
                tt, k = divmod(s, SIM)
                w = wcat[s % 2][:, 0:SW]
                nc.vector.scalar_tensor_tensor(
                    w, u1s, -BETA, Xr[:, :, tt, :], ALU.mult, ALU.add)

            def v2_step(s):
                tt, k = divmod(s, SIM)
                w = wcat[s % 2][:, SW:2 * SW]
                nc.vector.scalar_tensor_tensor(
                    w, u2s, -BETA, x2r[:, :, tt, k, :], ALU.mult, ALU.add)

            def u_reset(s, has1, has2):
                w = wcat[s % 2]
                if has1 and has2:
                    nc.vector.scalar_tensor_tensor(
                        ucat[:], w[:], THRESH, w[:], ALU.is_gt, ALU.subtract)
                elif has1:
                    nc.vector.scalar_tensor_tensor(
                        u1s, w[:, 0:SW], THRESH, w[:, 0:SW],
                        ALU.is_gt, ALU.subtract)
                elif has2:
                    nc.vector.scalar_tensor_tensor(
                        u2s, w[:, SW:2 * SW], THRESH, w[:, SW:2 * SW],
                        ALU.is_gt, ALU.subtract)

            def sig(s, has1, has2):
                """merged spike extraction for v1 (s1, chunks 0..NC1-1) and
                v2 (zb, chunks NC1..2*NC1-1) at the same step index"""
                tt, k = divmod(s, SIM)
                if has1 and has2:
                    nc.scalar.activation(
                        szr[:, :, tt, k, :],
                        wcat[s % 2].rearrange("p (c b) -> p c b", c=2 * NC1),
                        ACTF.Sigmoid, bias=bsig[:, 0:1], scale=SIGS)
                elif has1:
                    nc.scalar.activation(
                        szr[:, 0:NC1, tt, k, :],
                        wcat[s % 2][:, 0:SW].rearrange("p (c b) -> p c b",
                                                       c=NC1),
                        ACTF.Sigmoid, bias=bsig[:, 0:1], scale=SIGS)
                elif has2:
                    nc.scalar.activation(
                        szr[:, NC1:2 * NC1, tt, k, :],
                        wcat[s % 2][:, SW:2 * SW].rearrange("p (c b) -> p c b",
                                                            c=NC1),
                        ACTF.Sigmoid, bias=bsig[:, 0:1], scale=SIGS)

            def alif_front(s):
                """leak+input, margin d = w - thr, and the ACT spike
                extraction sa = sigmoid-sat(d) for alif step s; placed at
                the END of the outer step so the back ops of step s-1
                (emitted at the START) have a full step of slack."""
                tt, k = divmod(s, SIM)
                axap = axr[:, :, tt, k, :]
                saap = sar[:, :, tt, k, :]
                nc.vector.scalar_tensor_tensor(
                    va[:], va[:], BETA, axap, ALU.mult, ALU.add)
                nc.vector.tensor_tensor(dtl[:], va[:], thr[:], ALU.subtract)
                nc.scalar.activation(
                    saap, dtl[:], ACTF.Sigmoid, scale=SIGS)

            def alif_back(s):
                """soft reset via predicated copy (va = sa ? d : w, where
                d = w - thr is bit-exact w - sa*thr), adaptation update,
                and next-step thr."""
                tt, k = divmod(s, SIM)
                saap = sar[:, :, tt, k, :]
                # mask must be an int dtype; bf16 {0.0, 1.0} bitcast to
                # int16 keeps exact zero/nonzero semantics
                nc.vector.copy_predicated(
                    va[:], saap.bitcast(mybir.dt.int16), dtl[:])
                nc.vector.scalar_tensor_tensor(
                    ba[:], ba[:], BETA_B, saap, ALU.mult, ALU.add)
                # thr for the NEXT alif step (thr = 1 + rho*ba)
                nc.scalar.activation(
                    thr[:], ba[:], ACTF.Identity, bias=THRESH, scale=RHO)

            for s in range(NS + ALAG + 1):
                has1 = s1 is not None and s < NS
                has2 = x2 is not None and s < NS
                kA = s - ALAG        # alif front (w, d, sa) step
                kB = s - ALAG - 1    # alif back (reset, ba, thr) step
                hasA = saT is not None and 0 <= kA < NS
                hasB = saT is not None and 0 <= kB < NS
                if hasB:
                    alif_back(kB)
                if has1:
                    v1_step(s)
                if has2:
                    v2_step(s)
                u_reset(s, has1, has2)
                if hasA:
                    alif_front(kA)
                if has1 or has2:
                    sig(s, has1, has2)
            return sz

        def emit_l2(i):
            s1 = S1[i]
            ax = axpool.tile([128, NC2 * NKB], F32, tag="AX", name="AX")
            AXB[i] = ax
            for m2 in range(NC2):
                pms = [pmpool.tile([128, 512], F32, tag="pm", name="pm")
                       for _ in range(NKB // 512)]
                for ph, wp in enumerate((w2hp, w2lp)):
                    for c in range(NC1):
                        for n in range(NKB // 512):
                            nc.tensor.matmul(
                                pms[n][:],
                                wp[c][m2][:],
                                s1[:, c * NKB + n * 512:c * NKB + (n + 1) * 512],
                                start=(ph == 0 and c == 0),
                                stop=(ph == 1 and c == NC1 - 1))
                for n in range(NKB // 512):
                    nc.scalar.activation(
                        ax[:, m2 * NKB + n * 512:m2 * NKB + (n + 1) * 512],
                        pms[n][:], ACTF.Identity, bias=b2t[:, m2:m2 + 1])

        def emit_l3(i):
            s1 = S1[i]
            sa = SA[i]
            x2 = x2pool.tile([128, NC1 * NKB], F32, tag="X2", name="X2")
            X2B[i] = x2
            for m in range(NC1):
                pms = [pmpool.tile([128, 512], F32, tag="pm", name="pm")
                       for _ in range(NKB // 512)]
                for ph, wp in enumerate((w3hp, w3lp)):
                    for c in range(NC1):
                        for n in range(NKB // 512):
                            nc.tensor.matmul(
                                pms[n][:],
                                wp[c][m][:],
                                s1[:, c * NKB + n * 512:c * NKB + (n + 1) * 512],
                                start=(ph == 0 and c == 0), stop=False)
                    for c2 in range(NC2):
                        for n in range(NKB // 512):
                            nc.tensor.matmul(
                                pms[n][:],
                                wp[NC1 + c2][m][:],
                                sa[:, c2 * NKB + n * 512:c2 * NKB + (n + 1) * 512],
                                start=False,
                                stop=(ph == 1 and c2 == NC2 - 1))
                for n in range(NKB // 512):
                    nc.scalar.activation(
                        x2[:, m * NKB + n * 512:m * NKB + (n + 1) * 512],
                        pms[n][:], ACTF.Identity, bias=b3t[:, m:m + 1])

        def emit_l4(i, sz):
            t0 = i * TB
            zbr = sz.rearrange("p (c t k b) -> p c t k b", c=2 * NC1, t=TB,
                               k=SIM)
            po = popool.tile([NOUT, NB], F32, tag="po", name="po")
            first = True
            for k in range(SIM):
                for c in range(NC1):
                    nc.tensor.matmul(
                        po[:], w4fp[k][c][:], zbr[:, NC1 + c, :, k, :],
                        start=first,
                        stop=(k == SIM - 1 and c == NC1 - 1))
                    first = False
            ot = opool.tile([NOUT, NB], F32, tag="OT", name="OT")
            nc.scalar.activation(ot[:], po[:], ACTF.Identity, bias=b4t[:, 0:1])
            nc.sync.dma_start(
                out=bass.AP(outT, t0 * BC, [[T * BC, NOUT], [1, NB]]),
                in_=ot[:])

        # software-pipelined emission with 2-round skew; L1 of the NEXT
        # block leads each round so PE has boundary work
        emit_inp_l1(0)
        for fn in HEAVY:
            fn()
        for r in range(NBLK + 2):
            if r + 1 < NBLK:
                emit_inp_l1(r + 1)
            zb = emit_chains(r)
            if 1 <= r < NBLK + 1:
                emit_l3(r - 1)
            if r < NBLK:
                emit_l2(r)
            if r >= 2:
                emit_l4(r - 2, zb)

    return nc


def _prep_host(inputs):
    inp = np.ascontiguousarray(inputs["inp"], dtype=np.float32)
    W1 = np.asarray(inputs["W1"], np.float32)
    W2 = np.asarray(inputs["W2"], np.float32)
    W3 = np.asarray(inputs["W3"], np.float32)
    W4 = np.asarray(inputs["W4"], np.float32)
    b1 = np.asarray(inputs["b1"], np.float32)
    b2 = np.asarray(inputs["b2"], np.float32)
    b3 = np.asarray(inputs["b3"], np.float32)
    b4 = np.asarray(inputs["b4"], np.float32)

    def split(W):
        # [K, M] -> [K//128, M//128, 128, 128] contiguous blocks (FWL needs
        # contiguous weight tiles)
        WT = W.T
        K, M = WT.shape
        Wh = WT.astype(ml_dtypes.bfloat16)
        Wl = (WT - Wh.astype(np.float32)).astype(np.float16)
        def blk(A):
            return np.ascontiguousarray(
                A.reshape(K // 128, 128, M // 128, 128).transpose(0, 2, 1, 3))
        return blk(Wh), blk(Wl)

    w2h, w2l = split(W2)
    w3h, w3l = split(W3)
    # W4cat: per sim step k the output integrator weight is beta^(SIM-1-k)*W4
    W4T = W4.T.astype(np.float64)                     # [NS2, NOUT]
    w4cat = np.stack([(BETA ** (SIM - 1 - k)) * W4T for k in range(SIM)])
    w4cat = w4cat.reshape(SIM, NC1, 128, NOUT)
    w4f_ = w4cat.astype(np.float16)
    csum = float(sum(BETA ** k for k in range(SIM)))
    # b1 rides the L1 matmul as weight row NIN (ones appended to inpT)
    w1cat = np.vstack([W1.T, b1[None, :]])             # [NIN+1, NS1]
    shared = dict(
        w1t=np.ascontiguousarray(w1cat),
        w2h=w2h, w2l=w2l, w3h=w3h, w3l=w3l,
        w4f=np.ascontiguousarray(w4f_),
        b2m=np.ascontiguousarray(b2.reshape(NC2, 128)),
        b3m=np.ascontiguousarray(b3.reshape(NC1, 128)),
        b4c=np.ascontiguousarray((b4.astype(np.float64) * csum)
                                 .astype(np.float32).reshape(NOUT, 1)),
    )
    ones_row = np.ones((1, T, BC), np.float32)
    in_maps = []
    for c in range(NCORES):
        shard = inp[:, c * BC:(c + 1) * BC, :]                 # [T, BC, NIN]
        m = dict(shared)
        m["inpT"] = np.ascontiguousarray(
            np.concatenate([shard.transpose(2, 0, 1), ones_row], axis=0))
        in_maps.append(m)
    return in_maps


def run(inputs, trace=False, **kw):
    if "nc" not in _CACHE:
        _CACHE["nc"] = build_nc()
    nc = _CACHE["nc"]
    in_maps = _prep_host(inputs)
    res = run_bass_kernel_spmd(nc, in_maps, core_ids=list(range(NCORES)),
                               trace=trace, **kw)
    outs = []
    for c in range(NCORES):
        outT = res.results[c]["outT"]                          # [NOUT, T, BC]
        outs.append(np.ascontiguousarray(outT.transpose(1, 2, 0)))
    full = np.concatenate(outs, axis=1)                        # [T, B, NOUT]
    return full, res


def kernel(**inputs):
    out, _ = run(inputs)
    return out


# revision 53
# speedup vs baseline: 1.2755x; 1.1774x over previous
"""Trainium2 Bass kernel for the AdaptiveFF spiking network.

Sharding: data-parallel over batch, 8 NeuronCores, 32 batch elements per
core, weights replicated. No collectives needed.

Per-core kernel (all state feature-major: [feature_chunk=128, batch]):
  - X = inp @ W1.T is hoisted out of the sim loop (x_t is constant across
    the 4 sim steps) and batched per 8-step time block; b1 rides the
    matmul as a 701st ones-row, so the PSUM eviction is a bias-free copy
    that runs on the otherwise-idle GpSimd engine.
  - The three recurrences (LIF v1, ALIF va/ba, LIF v2) are emitted as
    per-step interleaved chains; ALIF lags 8 steps so the in-order
    engines always have independent work between the dependent ops of
    any one chain.
  - v1/v2 use a negated-state encoding u = s - w: the spike reset is a
    single fused STT (u = (w > 1) - w, exact fp32 parity with the
    reference) and the two chains' resets merge into one [128,256] op.
    v1 and v2 run at the same step index and their spike extractions
    merge into ONE saturated-sigmoid ACT op per step writing a shared
    s1|zb supertile (exact {0,1} outside ~1 ulp of threshold).
  - v1 and v2's leak+input STTs merge into ONE [128,256] op per step by
    co-locating X (k-replicated x4 at eviction) and x2 in a shared XX2
    supertile whose chunks address uniformly by (t, k).
  - ALIF stays entirely on the DVE (w STT, is_gt, sth, reset, ba; thr on
    ACT with a step of slack): any cross-engine hop inside this serial
    per-step loop adds its full round-trip latency to every step.
  - W2/W3 matmuls batch over (t, sim) with N=512 tiles and run as two
    passes: bf16 high + fp16 low residual, reproducing the fp32 product
    to ~1e-8 at 2x the fp32 rate. PSUM is evicted by ScalarE with the
    layer biases fused in.
  - The output integrator is folded into the W4 matmul by contracting
    over (sim_step, feature) against beta^(3-k)-scaled weight copies in
    a single fp16 pass (output-only path, no feedback).
Measured on trn2: baseline 1.283 ms; rel err ~0.015 vs the fp32 numpy
reference (the fp32 chaos floor of this spiking net is ~0.013-0.014).
"""

import sys

for p in ("/opt/trn_rl_repo", "/root/.axon_site/_ro/trn_rl_repo"):
    if p not in sys.path:
        sys.path.append(p)

from contextlib import ExitStack

import numpy as np
import ml_dtypes

from concourse import mybir
import concourse.bass as bass
import concourse.tile as tile
from concourse.tile import TileContext
from concourse.bass_utils import run_bass_kernel_spmd

F32 = mybir.dt.float32
BF16 = mybir.dt.bfloat16
F16 = mybir.dt.float16
ALU = mybir.AluOpType
ACTF = mybir.ActivationFunctionType

T, B, NIN = 200, 256, 700
NS1, NA, NS2, NOUT = 512, 256, 512, 20
SIM = 4
BETA, THRESH, BETA_B, RHO = 0.9, 1.0, 0.95, 0.5
NCORES = 8
BC = B // NCORES          # 32 batch per core
TB = 8                    # time-block
NBLK = T // TB
NC1 = NS1 // 128          # 4 feature chunks for s1/x2
NC2 = NA // 128           # 2 chunks for sa/ax
NINP = NIN + 1            # 701: b1 rides the L1 matmul as a ones-row
NCI = (NINP + 127) // 128  # 6 input chunks (last ragged: 61)
NB = TB * BC              # 256 (t, b) cols per block
NKB = TB * SIM * BC       # 1024 (t, k, b) cols per block

_CACHE = {}


def _split_waits(nc, max_waits=1):
    """walrus in this container rejects >1 sem-wait per instruction; hoist
    extras onto preceding InstEventSemaphore instructions on the same
    engine (program order makes them happen-before)."""
    for f in nc.m.functions:
        for bb in f.blocks:
            dirty = False
            newl = []
            for ins in bb.instructions:
                si = ins.sync_info
                if si is not None and len(si.on_wait) > max_waits:
                    waits = list(si.on_wait)
                    for w in waits[:-max_waits]:
                        ev = mybir.InstEventSemaphore(
                            name=nc.get_next_instruction_name(), ins=[], outs=[])
                        ev.engine = ins.engine
                        ev.sync_info = mybir.SyncInfo(on_wait=[w], on_update=[])
                        nc.register_instruction(ev, overwrite=True)
                        newl.append(ev)
                    ins.sync_info = mybir.SyncInfo(
                        on_wait=waits[-max_waits:], on_update=list(si.on_update))
                    dirty = True
                newl.append(ins)
            if dirty:
                bb.instructions = newl


def _patch_tile_drain():
    if getattr(tile.TileContext, "_wait_split_patched", False):
        return
    orig = tile.TileContext._drain_and_barrier

    def patched(self, tick_clock, wait_clock):
        orig(self, tick_clock, wait_clock)
        _split_waits(self.nc)

    tile.TileContext._drain_and_barrier = patched
    tile.TileContext._wait_split_patched = True


def build_nc():
    _patch_tile_drain()
    nc = bass.Bass("TRN2", target_bir_lowering=False)

    dp = nc.declare_dram_parameter
    inpT = dp("inpT", [NINP, T, BC], F32, isOutput=False)
    w1t = dp("w1t", [NINP, NS1], F32, isOutput=False)
    w2h = dp("w2h", [NC1, NA // 128, 128, 128], BF16, isOutput=False)
    w2l = dp("w2l", [NC1, NA // 128, 128, 128], F16, isOutput=False)
    w3h = dp("w3h", [NC1 + NC2, NC1, 128, 128], BF16, isOutput=False)
    w3l = dp("w3l", [NC1 + NC2, NC1, 128, 128], F16, isOutput=False)
    w4f = dp("w4f", [SIM, NC1, 128, NOUT], F16, isOutput=False)
    b2m = dp("b2m", [NC2, 128], F32, isOutput=False)
    b3m = dp("b3m", [NC1, 128], F32, isOutput=False)
    b4c = dp("b4c", [NOUT, 1], F32, isOutput=False)
    outT = dp("outT", [NOUT, T, BC], F32, isOutput=True)

    with TileContext(nc) as tc, ExitStack() as ctx:
        wpool = ctx.enter_context(tc.tile_pool(name="weights", bufs=1))
        spool = ctx.enter_context(tc.tile_pool(name="states", bufs=1))
        szpool = ctx.enter_context(tc.tile_pool(name="szbuf", bufs=3))
        sapool = ctx.enter_context(tc.tile_pool(name="sabuf", bufs=3))
        axpool = ctx.enter_context(tc.tile_pool(name="axbuf", bufs=2))
        xxpool = ctx.enter_context(tc.tile_pool(name="xxbuf", bufs=2))
        ipool = ctx.enter_context(tc.tile_pool(name="inp", bufs=2))
        opool = ctx.enter_context(tc.tile_pool(name="outt", bufs=2))
        pxpool = ctx.enter_context(tc.tile_pool(name="px", bufs=2, space="PSUM"))
        pmpool = ctx.enter_context(tc.tile_pool(name="pmid", bufs=5, space="PSUM"))
        popool = ctx.enter_context(tc.tile_pool(name="po", bufs=1, space="PSUM"))

        # ---- load weights ----
        w1 = []
        for c in range(NCI):
            kc = min(128, NINP - c * 128)
            wt = wpool.tile([kc, NS1], F32, tag=f"w1_{c}", name=f"w1_{c}")
            nc.sync.dma_start(out=wt[:], in_=w1t[c * 128:c * 128 + kc, :])
            w1.append(wt)

        def load_blocks(dram, nctot, nm, dt_, nm_name):
            # one DMA per K-chunk: [128, nm*128] tile whose m-th 128-col
            # slice is the contiguous [128,128] block (c, m)
            tiles = []
            for c in range(nctot):
                wt = wpool.tile([128, nm * 128], dt_, tag=f"{nm_name}_{c}",
                                name=f"{nm_name}_{c}")
                nc.sync.dma_start(
                    out=wt[:],
                    in_=bass.AP(dram, c * nm * 128 * 128,
                                [[128, 128], [128 * 128, nm], [1, 128]]))
                tiles.append([wt[:, m * 128:(m + 1) * 128] for m in range(nm)])
            return tiles

        w2hp, w3hp, w2lp, w3lp = [], [], [], []
        HEAVY = []
        HEAVY.append(lambda: w2hp.extend(load_blocks(w2h, NC1, NA // 128, BF16, "w2h")))
        HEAVY.append(lambda: w3hp.extend(load_blocks(w3h, NC1 + NC2, NC1, BF16, "w3h")))
        HEAVY.append(lambda: w2lp.extend(load_blocks(w2l, NC1, NA // 128, F16, "w2l")))
        HEAVY.append(lambda: w3lp.extend(load_blocks(w3l, NC1 + NC2, NC1, F16, "w3l")))
        w4fp = []

        def _load_w4():
            for k in range(SIM):
                rf = []
                for c in range(NC1):
                    wt = wpool.tile([128, NOUT], F16, tag=f"w4f_{k}_{c}",
                                    name=f"w4f_{k}_{c}")
                    nc.sync.dma_start(out=wt[:], in_=w4f[k, c, :, :])
                    rf.append(wt)
                w4fp.append(rf)
        HEAVY.append(_load_w4)
        b2t = wpool.tile([128, NC2], F32, tag="b2t", name="b2t")
        nc.sync.dma_start(out=b2t[:], in_=bass.AP(b2m, 0, [[1, 128], [128, NC2]]))
        b3t = wpool.tile([128, NC1], F32, tag="b3t", name="b3t")
        nc.sync.dma_start(out=b3t[:], in_=bass.AP(b3m, 0, [[1, 128], [128, NC1]]))
        b4t = wpool.tile([NOUT, 1], F32, tag="b4t", name="b4t")
        nc.sync.dma_start(out=b4t[:], in_=b4c[:, :])

        # ---- persistent states, layout [128, chunk*BC + b] ----
        SW = NC1 * BC
        AW = NC2 * BC
        wcat = [spool.tile([128, 2 * SW], F32, tag=f"wcat{j}", name=f"wcat{j}")
                for j in range(2)]
        ucat = spool.tile([128, 2 * SW], F32, tag="ucat", name="ucat")
        u1s = ucat[:, 0:SW]
        u2s = ucat[:, SW:2 * SW]
        va = spool.tile([128, AW], F32, tag="va", name="va")
        ba = spool.tile([128, AW], F32, tag="ba", name="ba")
        thr = spool.tile([128, AW], F32, tag="thr", name="thr")
        sth = spool.tile([128, AW], F32, tag="sth", name="sth")
        for st in (va, ba):
            nc.vector.memset(st[:], 0.0)
        nc.vector.memset(ucat[:], 0.0)

        # sigmoid-spike bias tile: s = sigmoid(SIGS*(v - THRESH)) saturates
        # to exact {0,1} outside ~1 ulp of the threshold
        SIGS = 1e8
        bsig = wpool.tile([128, 1], F32, tag="bsig", name="bsig")
        nc.vector.memset(bsig[:], -SIGS * THRESH)

        # per-block tiles carried between skewed emission rounds
        S1 = [None] * NBLK
        SA = [None] * NBLK
        AXB = [None] * NBLK
        # XX2[j]: [X(j) k-replicated x4 (chunks 0..NC1-1) | x2(j-2)
        # (chunks NC1..2*NC1-1)] supertile read by chains(j); both writers
        # (L1(j) evict, L3(j-2) evict) run in round j-1, so the v1 and v2
        # leak+input STTs merge into one [128,256] op per step.
        XX2 = [None] * (NBLK + 2)

        def alloc_xx2(j):
            if 0 <= j < NBLK + 2 and XX2[j] is None:
                XX2[j] = xxpool.tile([128, 2 * NC1 * NKB], F32, tag="XX2",
                                     name="XX2")

        def emit_inp_l1(i):
            """inp DMA, L1 matmuls (b1 rides as the 701st ones-row), X
            eviction k-replicated x4 into XX2[i] chunks 0..NC1-1 as
            bias-free ACT copies."""
            t0 = i * TB
            itiles = []
            for c in range(NCI):
                kc = min(128, NINP - c * 128)
                it = ipool.tile([kc, NB], F32, tag=f"inp_{c}", name=f"inp_{c}")
                nc.sync.dma_start(
                    out=it[:],
                    in_=bass.AP(inpT, c * 128 * T * BC + t0 * BC,
                                [[T * BC, kc], [1, NB]]))
                itiles.append(it)
            px = [pxpool.tile([128, 2 * NB], F32, tag="px", name="px")
                  for _ in range(2)]
            for mt in range(2):
                for m2 in range(2):
                    m = 2 * mt + m2
                    for c in range(NCI):
                        nc.tensor.matmul(
                            px[mt][:, m2 * NB:(m2 + 1) * NB],
                            w1[c][:, m * 128:(m + 1) * 128],
                            itiles[c][:],
                            start=(c == 0), stop=(c == NCI - 1))
            xxr = XX2[i].rearrange("p (c t k b) -> p c t k b", c=2 * NC1,
                                   t=TB, k=SIM)
            for mt in range(2):
                pxr = px[mt].rearrange("p (m t b) -> p m t b", m=2, t=TB)
                for k in range(SIM):
                    nc.scalar.activation(
                        xxr[:, 2 * mt:2 * mt + 2, :, k, :], pxr[:],
                        ACTF.Identity)

        def emit_chains(i):
            """Interleaved per-step emission of the three state chains:
            v1/s1 of block i, ALIF of block i-1 (lagged 8 steps so the
            L2(i-1) psum evictions land first), v2/s2/z of block i-2 (same
            step index as v1, so both spike extractions merge into one ACT
            op). Interleaving keeps the in-order DVE busy during the ACT
            round-trips of each individual chain."""
            NS = TB * SIM
            ALAG = 8
            s1 = saT = x2 = sz = None
            szr = axr = sar = None
            has1 = 0 <= i < NBLK
            has2 = 0 <= i - 2 < NBLK
            if has1 or has2:
                # spike supertile: chunks 0..NC1-1 hold s1 of block i,
                # chunks NC1..2*NC1-1 hold s2 (zb) of block i-2
                sz = szpool.tile([128, 2 * NC1 * NKB], BF16, tag="SZ",
                                 name="SZ")
                szr = sz.rearrange("p (c t k b) -> p c t k b", c=2 * NC1,
                                   t=TB, k=SIM)
            if has1:
                s1 = sz
                S1[i] = sz
            if 0 <= i - 1 < NBLK:
                ax = AXB[i - 1]
                saT = sapool.tile([128, NC2 * NKB], BF16, tag="SA", name="SA")
                SA[i - 1] = saT
                axr = ax.rearrange("p (c t k b) -> p c t k b", c=NC2, t=TB, k=SIM)
                sar = saT.rearrange("p (c t k b) -> p c t k b", c=NC2, t=TB, k=SIM)
                nc.scalar.activation(
                    thr[:], ba[:], ACTF.Identity, bias=THRESH, scale=RHO)
            if has2:
                x2 = XX2[i]
            xxr = XX2[i].rearrange("p (c t k b) -> p c t k b", c=2 * NC1,
                                   t=TB, k=SIM)
            # merged v1+v2 leak+input: one STT over the contiguous state
            # range, reading [X | x2] chunks of the XX2 supertile
            if has1 and has2:
                wlo, whi, clo, chi = 0, 2 * SW, 0, 2 * NC1
            elif has1:
                wlo, whi, clo, chi = 0, SW, 0, NC1
            else:
                wlo, whi, clo, chi = SW, 2 * SW, NC1, 2 * NC1

            def v12_step(s):
                tt, k = divmod(s, SIM)
                nc.vector.scalar_tensor_tensor(
                    wcat[s % 2][:, wlo:whi], ucat[:, wlo:whi], -BETA,
                    xxr[:, clo:chi, tt, k, :], ALU.mult, ALU.add)

            def u_reset(s, has1, has2):
                w = wcat[s % 2]
                if has1 and has2:
                    nc.vector.scalar_tensor_tensor(
                        ucat[:], w[:], THRESH, w[:], ALU.is_gt, ALU.subtract)
                elif has1:
                    nc.vector.scalar_tensor_tensor(
                        u1s, w[:, 0:SW], THRESH, w[:, 0:SW],
                        ALU.is_gt, ALU.subtract)
                elif has2:
                    nc.vector.scalar_tensor_tensor(
                        u2s, w[:, SW:2 * SW], THRESH, w[:, SW:2 * SW],
                        ALU.is_gt, ALU.subtract)

            def sig(s, has1, has2):
                """merged spike extraction for v1 (s1, chunks 0..NC1-1) and
                v2 (zb, chunks NC1..2*NC1-1) at the same step index"""
                tt, k = divmod(s, SIM)
                if has1 and has2:
                    nc.scalar.activation(
                        szr[:, :, tt, k, :],
                        wcat[s % 2].rearrange("p (c b) -> p c b", c=2 * NC1),
                        ACTF.Sigmoid, bias=bsig[:, 0:1], scale=SIGS)
                elif has1:
                    nc.scalar.activation(
                        szr[:, 0:NC1, tt, k, :],
                        wcat[s % 2][:, 0:SW].rearrange("p (c b) -> p c b",
                                                       c=NC1),
                        ACTF.Sigmoid, bias=bsig[:, 0:1], scale=SIGS)
                elif has2:
                    nc.scalar.activation(
                        szr[:, NC1:2 * NC1, tt, k, :],
                        wcat[s % 2][:, SW:2 * SW].rearrange("p (c b) -> p c b",
                                                            c=NC1),
                        ACTF.Sigmoid, bias=bsig[:, 0:1], scale=SIGS)

            def alif_step(s):
                tt, k = divmod(s, SIM)
                axap = axr[:, :, tt, k, :]
                saap = sar[:, :, tt, k, :]
                nc.vector.scalar_tensor_tensor(
                    va[:], va[:], BETA, axap, ALU.mult, ALU.add)
                nc.vector.tensor_tensor(saap, va[:], thr[:], ALU.is_gt)
                nc.vector.tensor_tensor(sth[:], saap, thr[:], ALU.mult)

            def alif_reset(s):
                tt, k = divmod(s, SIM)
                saap = sar[:, :, tt, k, :]
                nc.vector.tensor_tensor(va[:], va[:], sth[:], ALU.subtract)
                nc.vector.scalar_tensor_tensor(
                    ba[:], ba[:], BETA_B, saap, ALU.mult, ALU.add)
                # thr for the NEXT alif step, computed on ACT with a full
                # step of slack (thr = 1 + rho*ba)
                nc.scalar.activation(
                    thr[:], ba[:], ACTF.Identity, bias=THRESH, scale=RHO)

            for s in range(NS + ALAG):
                hasv = (has1 or has2) and s < NS
                if hasv:
                    v12_step(s)
                if saT is not None and ALAG <= s < NS + ALAG:
                    alif_step(s - ALAG)
                if hasv:
                    u_reset(s, has1, has2)
                if saT is not None and ALAG <= s < NS + ALAG:
                    alif_reset(s - ALAG)
                if hasv:
                    sig(s, has1, has2)
            return sz

        def emit_l2(i):
            s1 = S1[i]
            ax = axpool.tile([128, NC2 * NKB], F32, tag="AX", name="AX")
            AXB[i] = ax
            for m2 in range(NC2):
                pms = [pmpool.tile([128, 512], F32, tag="pm", name="pm")
                       for _ in range(NKB // 512)]
                for ph, wp in enumerate((w2hp, w2lp)):
                    for c in range(NC1):
                        for n in range(NKB // 512):
                            nc.tensor.matmul(
                                pms[n][:],
                                wp[c][m2][:],
                                s1[:, c * NKB + n * 512:c * NKB + (n + 1) * 512],
                                start=(ph == 0 and c == 0),
                                stop=(ph == 1 and c == NC1 - 1))
                for n in range(NKB // 512):
                    nc.scalar.activation(
                        ax[:, m2 * NKB + n * 512:m2 * NKB + (n + 1) * 512],
                        pms[n][:], ACTF.Identity, bias=b2t[:, m2:m2 + 1])

        def emit_l3(i):
            # x2(i) lands in XX2[i+2] chunks NC1..2*NC1-1
            s1 = S1[i]
            sa = SA[i]
            x2 = XX2[i + 2]
            for m in range(NC1):
                pms = [pmpool.tile([128, 512], F32, tag="pm", name="pm")
                       for _ in range(NKB // 512)]
                for ph, wp in enumerate((w3hp, w3lp)):
                    for c in range(NC1):
                        for n in range(NKB // 512):
                            nc.tensor.matmul(
                                pms[n][:],
                                wp[c][m][:],
                                s1[:, c * NKB + n * 512:c * NKB + (n + 1) * 512],
                                start=(ph == 0 and c == 0), stop=False)
                    for c2 in range(NC2):
                        for n in range(NKB // 512):
                            nc.tensor.matmul(
                                pms[n][:],
                                wp[NC1 + c2][m][:],
                                sa[:, c2 * NKB + n * 512:c2 * NKB + (n + 1) * 512],
                                start=False,
                                stop=(ph == 1 and c2 == NC2 - 1))
                base = (NC1 + m) * NKB
                for n in range(NKB // 512):
                    nc.scalar.activation(
                        x2[:, base + n * 512:base + (n + 1) * 512],
                        pms[n][:], ACTF.Identity, bias=b3t[:, m:m + 1])

        def emit_l4(i, sz):
            t0 = i * TB
            zbr = sz.rearrange("p (c t k b) -> p c t k b", c=2 * NC1, t=TB,
                               k=SIM)
            po = popool.tile([NOUT, NB], F32, tag="po", name="po")
            first = True
            for k in range(SIM):
                for c in range(NC1):
                    nc.tensor.matmul(
                        po[:], w4fp[k][c][:], zbr[:, NC1 + c, :, k, :],
                        start=first,
                        stop=(k == SIM - 1 and c == NC1 - 1))
                    first = False
            ot = opool.tile([NOUT, NB], F32, tag="OT", name="OT")
            nc.scalar.activation(ot[:], po[:], ACTF.Identity, bias=b4t[:, 0:1])
            nc.sync.dma_start(
                out=bass.AP(outT, t0 * BC, [[T * BC, NOUT], [1, NB]]),
                in_=ot[:])

        # software-pipelined emission with 2-round skew; L1 of the NEXT
        # block leads each round so PE has boundary work.  XX2[r+1] is
        # allocated one round ahead: its writers (L1(r+1) and L3(r-1)
        # evictions) run during round r.
        alloc_xx2(0)
        emit_inp_l1(0)
        for fn in HEAVY:
            fn()
        for r in range(NBLK + 2):
            alloc_xx2(r + 1)
            if r + 1 < NBLK:
                emit_inp_l1(r + 1)
            zb = emit_chains(r)
            if 1 <= r < NBLK + 1:
                emit_l3(r - 1)
            if r < NBLK:
                emit_l2(r)
            if r >= 2:
                emit_l4(r - 2, zb)

    return nc


def _prep_host(inputs):
    inp = np.ascontiguousarray(inputs["inp"], dtype=np.float32)
    W1 = np.asarray(inputs["W1"], np.float32)
    W2 = np.asarray(inputs["W2"], np.float32)
    W3 = np.asarray(inputs["W3"], np.float32)
    W4 = np.asarray(inputs["W4"], np.float32)
    b1 = np.asarray(inputs["b1"], np.float32)
    b2 = np.asarray(inputs["b2"], np.float32)
    b3 = np.asarray(inputs["b3"], np.float32)
    b4 = np.asarray(inputs["b4"], np.float32)

    def split(W):
        # [K, M] -> [K//128, M//128, 128, 128] contiguous blocks (FWL needs
        # contiguous weight tiles)
        WT = W.T
        K, M = WT.shape
        Wh = WT.astype(ml_dtypes.bfloat16)
        Wl = (WT - Wh.astype(np.float32)).astype(np.float16)
        def blk(A):
            return np.ascontiguousarray(
                A.reshape(K // 128, 128, M // 128, 128).transpose(0, 2, 1, 3))
        return blk(Wh), blk(Wl)

    w2h, w2l = split(W2)
    w3h, w3l = split(W3)
    # W4cat: per sim step k the output integrator weight is beta^(SIM-1-k)*W4
    W4T = W4.T.astype(np.float64)                     # [NS2, NOUT]
    w4cat = np.stack([(BETA ** (SIM - 1 - k)) * W4T for k in range(SIM)])
    w4cat = w4cat.reshape(SIM, NC1, 128, NOUT)
    w4f_ = w4cat.astype(np.float16)
    csum = float(sum(BETA ** k for k in range(SIM)))
    # b1 rides the L1 matmul as weight row NIN (ones appended to inpT)
    w1cat = np.vstack([W1.T, b1[None, :]])             # [NIN+1, NS1]
    shared = dict(
        w1t=np.ascontiguousarray(w1cat),
        w2h=w2h, w2l=w2l, w3h=w3h, w3l=w3l,
        w4f=np.ascontiguousarray(w4f_),
        b2m=np.ascontiguousarray(b2.reshape(NC2, 128)),
        b3m=np.ascontiguousarray(b3.reshape(NC1, 128)),
        b4c=np.ascontiguousarray((b4.astype(np.float64) * csum)
                                 .astype(np.float32).reshape(NOUT, 1)),
    )
    ones_row = np.ones((1, T, BC), np.float32)
    in_maps = []
    for c in range(NCORES):
        shard = inp[:, c * BC:(c + 1) * BC, :]                 # [T, BC, NIN]
        m = dict(shared)
        m["inpT"] = np.ascontiguousarray(
            np.concatenate([shard.transpose(2, 0, 1), ones_row], axis=0))
        in_maps.append(m)
    return in_maps


def run(inputs, trace=False, **kw):
    if "nc" not in _CACHE:
        _CACHE["nc"] = build_nc()
    nc = _CACHE["nc"]
    in_maps = _prep_host(inputs)
    res = run_bass_kernel_spmd(nc, in_maps, core_ids=list(range(NCORES)),
                               trace=trace, **kw)
    outs = []
    for c in range(NCORES):
        outT = res.results[c]["outT"]                          # [NOUT, T, BC]
        outs.append(np.ascontiguousarray(outT.transpose(1, 2, 0)))
    full = np.concatenate(outs, axis=1)                        # [T, B, NOUT]
    return full, res


def kernel(**inputs):
    out, _ = run(inputs)
    return out


# revision 55
# speedup vs baseline: 1.3559x; 1.0630x over previous
"""Trainium2 Bass kernel for the AdaptiveFF spiking network.  (B1 state)

Sharding: data-parallel over batch, 8 NeuronCores, 32 batch elements per
core, weights replicated. No collectives needed.

Per-core kernel (all state feature-major: [feature_chunk=128, batch]):
  - X = inp @ W1.T is hoisted out of the sim loop (x_t is constant across
    the 4 sim steps) and batched per 8-step time block (fp32 matmuls).
  - The three recurrences (LIF v1, ALIF va/ba, LIF v2) are emitted as
    per-step interleaved DVE chains of three different time blocks
    (2-round software-pipeline skew; ALIF lags 8 steps), so the in-order
    engines always have independent work between the dependent ops of
    any one chain.
  - v1/v2 use a negated-state encoding u = s - w: the spike reset is a
    single fused STT (u = (w > 1) - w, exact fp32 parity with the
    reference) and the two chains' resets merge into one [128,256] op.
    v1 and v2 run at the same step index and their spike extractions
    merge into ONE saturated-sigmoid ACT op per step writing a shared
    s1|zb supertile (exact {0,1} outside ~1 ulp of threshold).
  - W2/W3 matmuls batch over (t, sim) with N=512 tiles and run as two
    passes: bf16 high + fp16 low residual, reproducing the fp32 product
    to ~1e-8 at 2x the fp32 rate. PSUM is evicted by ScalarE with the
    layer biases fused in.
  - The output integrator is folded into the W4 matmul by contracting
    over (sim_step, feature) against beta^(3-k)-scaled weight copies in
    a single fp16 pass (output-only path, no feedback).
Measured on trn2: ~1.274 ms HW exec, rel err ~0.0152 vs the fp32 numpy
reference (the fp32 chaos floor of this spiking net is ~0.013-0.014).
"""

import sys

for p in ("/opt/trn_rl_repo", "/root/.axon_site/_ro/trn_rl_repo"):
    if p not in sys.path:
        sys.path.append(p)

from contextlib import ExitStack

import numpy as np
import ml_dtypes

from concourse import mybir
import concourse.bass as bass
import concourse.tile as tile
from concourse.tile import TileContext
from concourse.bass_utils import run_bass_kernel_spmd

F32 = mybir.dt.float32
BF16 = mybir.dt.bfloat16
F16 = mybir.dt.float16
ALU = mybir.AluOpType
ACTF = mybir.ActivationFunctionType

T, B, NIN = 200, 256, 700
NS1, NA, NS2, NOUT = 512, 256, 512, 20
SIM = 4
BETA, THRESH, BETA_B, RHO = 0.9, 1.0, 0.95, 0.5
NCORES = 8
BC = B // NCORES          # 32 batch per core
TB = 8                    # time-block
NBLK = T // TB
NC1 = NS1 // 128          # 4 feature chunks for s1/x2
NC2 = NA // 128           # 2 chunks for sa/ax
NCI = (NIN + 127) // 128  # 6 input chunks (last ragged: 60)
NB = TB * BC              # 256 (t, b) cols per block
NKB = TB * SIM * BC       # 1024 (t, k, b) cols per block

_CACHE = {}


def _split_waits(nc, max_waits=1):
    """walrus in this container rejects >1 sem-wait per instruction; hoist
    extras onto preceding InstEventSemaphore instructions on the same
    engine (program order makes them happen-before)."""
    for f in nc.m.functions:
        for bb in f.blocks:
            dirty = False
            newl = []
            for ins in bb.instructions:
                si = ins.sync_info
                if si is not None and len(si.on_wait) > max_waits:
                    waits = list(si.on_wait)
                    for w in waits[:-max_waits]:
                        ev = mybir.InstEventSemaphore(
                            name=nc.get_next_instruction_name(), ins=[], outs=[])
                        ev.engine = ins.engine
                        ev.sync_info = mybir.SyncInfo(on_wait=[w], on_update=[])
                        nc.register_instruction(ev, overwrite=True)
                        newl.append(ev)
                    ins.sync_info = mybir.SyncInfo(
                        on_wait=waits[-max_waits:], on_update=list(si.on_update))
                    dirty = True
                newl.append(ins)
            if dirty:
                bb.instructions = newl


def _patch_tile_drain():
    if getattr(tile.TileContext, "_wait_split_patched", False):
        return
    orig = tile.TileContext._drain_and_barrier

    def patched(self, tick_clock, wait_clock):
        orig(self, tick_clock, wait_clock)
        _split_waits(self.nc)

    tile.TileContext._drain_and_barrier = patched
    tile.TileContext._wait_split_patched = True


def build_nc():
    _patch_tile_drain()
    nc = bass.Bass("TRN2", target_bir_lowering=False)

    dp = nc.declare_dram_parameter
    inpT = dp("inpT", [NIN, T, BC], F32, isOutput=False)
    w1t = dp("w1t", [NIN, NS1], F32, isOutput=False)
    w2h = dp("w2h", [NC1, NA // 128, 128, 128], BF16, isOutput=False)
    w2l = dp("w2l", [NC1, NA // 128, 128, 128], F16, isOutput=False)
    w3h = dp("w3h", [NC1 + NC2, NC1, 128, 128], BF16, isOutput=False)
    w3l = dp("w3l", [NC1 + NC2, NC1, 128, 128], F16, isOutput=False)
    w4f = dp("w4f", [SIM, NC1, 128, NOUT], F16, isOutput=False)
    b1m = dp("b1m", [NC1, 128], F32, isOutput=False)
    b2m = dp("b2m", [NC2, 128], F32, isOutput=False)
    b3m = dp("b3m", [NC1, 128], F32, isOutput=False)
    b4c = dp("b4c", [NOUT, 1], F32, isOutput=False)
    outT = dp("outT", [NOUT, T, BC], F32, isOutput=True)

    with TileContext(nc) as tc, ExitStack() as ctx:
        wpool = ctx.enter_context(tc.tile_pool(name="weights", bufs=1))
        spool = ctx.enter_context(tc.tile_pool(name="states", bufs=1))
        xpool = ctx.enter_context(tc.tile_pool(name="xbuf", bufs=3))
        szpool = ctx.enter_context(tc.tile_pool(name="szbuf", bufs=3))
        sapool = ctx.enter_context(tc.tile_pool(name="sabuf", bufs=3))
        axpool = ctx.enter_context(tc.tile_pool(name="axbuf", bufs=3))
        x2pool = ctx.enter_context(tc.tile_pool(name="x2buf", bufs=3))
        ipool = ctx.enter_context(tc.tile_pool(name="inp", bufs=2))
        opool = ctx.enter_context(tc.tile_pool(name="outt", bufs=2))
        pxpool = ctx.enter_context(tc.tile_pool(name="px", bufs=2, space="PSUM"))
        pmpool = ctx.enter_context(tc.tile_pool(name="pmid", bufs=5, space="PSUM"))
        popool = ctx.enter_context(tc.tile_pool(name="po", bufs=1, space="PSUM"))

        # ---- load weights ----
        w1 = []
        for c in range(NCI):
            kc = min(128, NIN - c * 128)
            wt = wpool.tile([kc, NS1], F32, tag=f"w1_{c}", name=f"w1_{c}")
            nc.sync.dma_start(out=wt[:], in_=w1t[c * 128:c * 128 + kc, :])
            w1.append(wt)

        def load_blocks(dram, nctot, nm, dt_, nm_name):
            # one DMA per K-chunk: [128, nm*128] tile whose m-th 128-col
            # slice is the contiguous [128,128] block (c, m)
            tiles = []
            for c in range(nctot):
                wt = wpool.tile([128, nm * 128], dt_, tag=f"{nm_name}_{c}",
                                name=f"{nm_name}_{c}")
                nc.sync.dma_start(
                    out=wt[:],
                    in_=bass.AP(dram, c * nm * 128 * 128,
                                [[128, 128], [128 * 128, nm], [1, 128]]))
                tiles.append([wt[:, m * 128:(m + 1) * 128] for m in range(nm)])
            return tiles

        w2hp, w3hp, w2lp, w3lp = [], [], [], []
        HEAVY = []
        HEAVY.append(lambda: w2hp.extend(load_blocks(w2h, NC1, NA // 128, BF16, "w2h")))
        HEAVY.append(lambda: w3hp.extend(load_blocks(w3h, NC1 + NC2, NC1, BF16, "w3h")))
        HEAVY.append(lambda: w2lp.extend(load_blocks(w2l, NC1, NA // 128, F16, "w2l")))
        HEAVY.append(lambda: w3lp.extend(load_blocks(w3l, NC1 + NC2, NC1, F16, "w3l")))
        w4fp = []

        def _load_w4():
            for k in range(SIM):
                rf = []
                for c in range(NC1):
                    wt = wpool.tile([128, NOUT], F16, tag=f"w4f_{k}_{c}",
                                    name=f"w4f_{k}_{c}")
                    nc.sync.dma_start(out=wt[:], in_=w4f[k, c, :, :])
                    rf.append(wt)
                w4fp.append(rf)
        HEAVY.append(_load_w4)
        b1t = wpool.tile([128, NC1], F32, tag="b1t", name="b1t")
        nc.sync.dma_start(out=b1t[:], in_=bass.AP(b1m, 0, [[1, 128], [128, NC1]]))
        b2t = wpool.tile([128, NC2], F32, tag="b2t", name="b2t")
        nc.sync.dma_start(out=b2t[:], in_=bass.AP(b2m, 0, [[1, 128], [128, NC2]]))
        b3t = wpool.tile([128, NC1], F32, tag="b3t", name="b3t")
        nc.sync.dma_start(out=b3t[:], in_=bass.AP(b3m, 0, [[1, 128], [128, NC1]]))
        b4t = wpool.tile([NOUT, 1], F32, tag="b4t", name="b4t")
        nc.sync.dma_start(out=b4t[:], in_=b4c[:, :])

        # ---- persistent states, layout [128, chunk*BC + b] ----
        SW = NC1 * BC
        AW = NC2 * BC
        wcat = [spool.tile([128, 2 * SW], F32, tag=f"wcat{j}", name=f"wcat{j}")
                for j in range(2)]
        ucat = spool.tile([128, 2 * SW], F32, tag="ucat", name="ucat")
        u1s = ucat[:, 0:SW]
        u2s = ucat[:, SW:2 * SW]
        va = spool.tile([128, AW], F32, tag="va", name="va")
        ba = spool.tile([128, AW], F32, tag="ba", name="ba")
        thr = spool.tile([128, AW], F32, tag="thr", name="thr")
        sth = spool.tile([128, AW], F32, tag="sth", name="sth")
        for st in (va, ba):
            nc.vector.memset(st[:], 0.0)
        nc.vector.memset(ucat[:], 0.0)

        # sigmoid-spike bias tile: s = sigmoid(SIGS*(v - THRESH)) saturates
        # to exact {0,1} outside ~1 ulp of the threshold
        SIGS = 1e8
        bsig = wpool.tile([128, 1], F32, tag="bsig", name="bsig")
        nc.vector.memset(bsig[:], -SIGS * THRESH)

        # per-block tiles carried between skewed emission rounds
        S1 = [None] * NBLK
        SA = [None] * NBLK
        AXB = [None] * NBLK
        X2B = [None] * NBLK
        XS = [None] * NBLK

        def emit_inp_l1(i):
            """inp DMA, L1 matmuls, X eviction for block i."""
            t0 = i * TB
            itiles = []
            for c in range(NCI):
                kc = min(128, NIN - c * 128)
                it = ipool.tile([kc, NB], F32, tag=f"inp_{c}", name=f"inp_{c}")
                nc.sync.dma_start(
                    out=it[:],
                    in_=bass.AP(inpT, c * 128 * T * BC + t0 * BC,
                                [[T * BC, kc], [1, NB]]))
                itiles.append(it)
            px = [pxpool.tile([128, 2 * NB], F32, tag="px", name="px")
                  for _ in range(2)]
            for mt in range(2):
                for m2 in range(2):
                    m = 2 * mt + m2
                    for c in range(NCI):
                        nc.tensor.matmul(
                            px[mt][:, m2 * NB:(m2 + 1) * NB],
                            w1[c][:, m * 128:(m + 1) * 128],
                            itiles[c][:],
                            start=(c == 0), stop=(c == NCI - 1))
            X = xpool.tile([128, NC1 * NB], F32, tag="X", name="X")
            XS[i] = X
            for m in range(NC1):
                nc.scalar.activation(
                    X[:, m * NB:(m + 1) * NB],
                    px[m // 2][:, (m % 2) * NB:(m % 2 + 1) * NB],
                    ACTF.Identity, bias=b1t[:, m:m + 1])

        def emit_chains(i):
            """Interleaved per-step emission of the three state chains:
            v1/s1 of block i, ALIF of block i-1 (lagged 8 steps so the
            L2(i-1) psum evictions land first), v2/s2/z of block i-2 (same
            step index as v1, so both spike extractions merge into one ACT
            op)."""
            NS = TB * SIM
            ALAG = 8
            s1 = saT = x2 = sz = None
            Xr = szr = axr = sar = x2r = None
            if 0 <= i < NBLK or 0 <= i - 2 < NBLK:
                # spike supertile: chunks 0..NC1-1 hold s1 of block i,
                # chunks NC1..2*NC1-1 hold s2 (zb) of block i-2
                sz = szpool.tile([128, 2 * NC1 * NKB], BF16, tag="SZ",
                                 name="SZ")
                szr = sz.rearrange("p (c t k b) -> p c t k b", c=2 * NC1,
                                   t=TB, k=SIM)
            if 0 <= i < NBLK:
                s1 = sz
                S1[i] = sz
                Xr = XS[i].rearrange("p (m t b) -> p m t b", m=NC1, t=TB)
            if 0 <= i - 1 < NBLK:
                ax = AXB[i - 1]
                saT = sapool.tile([128, NC2 * NKB], BF16, tag="SA", name="SA")
                SA[i - 1] = saT
                axr = ax.rearrange("p (c t k b) -> p c t k b", c=NC2, t=TB, k=SIM)
                sar = saT.rearrange("p (c t k b) -> p c t k b", c=NC2, t=TB, k=SIM)
                nc.scalar.activation(
                    thr[:], ba[:], ACTF.Identity, bias=THRESH, scale=RHO)
            if 0 <= i - 2 < NBLK:
                x2 = X2B[i - 2]
                x2r = x2.rearrange("p (c t k b) -> p c t k b", c=NC1, t=TB, k=SIM)

            def v1_step(s):
                tt, k = divmod(s, SIM)
                w = wcat[s % 2][:, 0:SW]
                nc.vector.scalar_tensor_tensor(
                    w, u1s, -BETA, Xr[:, :, tt, :], ALU.mult, ALU.add)

            def v2_step(s):
                tt, k = divmod(s, SIM)
                w = wcat[s % 2][:, SW:2 * SW]
                nc.vector.scalar_tensor_tensor(
                    w, u2s, -BETA, x2r[:, :, tt, k, :], ALU.mult, ALU.add)

            def u_reset(s, has1, has2):
                w = wcat[s % 2]
                if has1 and has2:
                    nc.vector.scalar_tensor_tensor(
                        ucat[:], w[:], THRESH, w[:], ALU.is_gt, ALU.subtract)
                elif has1:
                    nc.vector.scalar_tensor_tensor(
                        u1s, w[:, 0:SW], THRESH, w[:, 0:SW],
                        ALU.is_gt, ALU.subtract)
                elif has2:
                    nc.vector.scalar_tensor_tensor(
                        u2s, w[:, SW:2 * SW], THRESH, w[:, SW:2 * SW],
                        ALU.is_gt, ALU.subtract)

            def sig(s, has1, has2):
                """merged spike extraction for v1 (s1, chunks 0..NC1-1) and
                v2 (zb, chunks NC1..2*NC1-1) at the same step index"""
                tt, k = divmod(s, SIM)
                if has1 and has2:
                    nc.scalar.activation(
                        szr[:, :, tt, k, :],
                        wcat[s % 2].rearrange("p (c b) -> p c b", c=2 * NC1),
                        ACTF.Sigmoid, bias=bsig[:, 0:1], scale=SIGS)
                elif has1:
                    nc.scalar.activation(
                        szr[:, 0:NC1, tt, k, :],
                        wcat[s % 2][:, 0:SW].rearrange("p (c b) -> p c b",
                                                       c=NC1),
                        ACTF.Sigmoid, bias=bsig[:, 0:1], scale=SIGS)
                elif has2:
                    nc.scalar.activation(
                        szr[:, NC1:2 * NC1, tt, k, :],
                        wcat[s % 2][:, SW:2 * SW].rearrange("p (c b) -> p c b",
                                                            c=NC1),
                        ACTF.Sigmoid, bias=bsig[:, 0:1], scale=SIGS)

            def alif_w(s):
                tt, k = divmod(s, SIM)
                nc.vector.scalar_tensor_tensor(
                    va[:], va[:], BETA, axr[:, :, tt, k, :],
                    ALU.mult, ALU.add)

            def alif_sa(s):
                tt, k = divmod(s, SIM)
                nc.vector.tensor_tensor(
                    sar[:, :, tt, k, :], va[:], thr[:], ALU.is_gt)

            def alif_sth(s):
                tt, k = divmod(s, SIM)
                nc.vector.tensor_tensor(
                    sth[:], sar[:, :, tt, k, :], thr[:], ALU.mult)

            def alif_ba(s):
                tt, k = divmod(s, SIM)
                nc.vector.scalar_tensor_tensor(
                    ba[:], ba[:], BETA_B, sar[:, :, tt, k, :],
                    ALU.mult, ALU.add)
                # thr for the NEXT alif step, computed on ACT with a full
                # step of slack (thr = 1 + rho*ba)
                nc.scalar.activation(
                    thr[:], ba[:], ACTF.Identity, bias=THRESH, scale=RHO)

            def alif_sub(s):
                nc.vector.tensor_tensor(va[:], va[:], sth[:], ALU.subtract)

            # DVE order per step: v1w, aw, v2w, asa, u, asth, aba, asub --
            # every dependent pair (aw->asa, asa->asth, asth->asub,
            # asub->aw', u->v1w') is separated by at least one independent
            # op, so the in-order DVE never exposes a write->read SBUF
            # round-trip between back-to-back instructions.
            for s in range(NS + ALAG):
                has1 = s1 is not None and s < NS
                has2 = x2 is not None and s < NS
                hasA = saT is not None and ALAG <= s < NS + ALAG
                k = s - ALAG
                if has1:
                    v1_step(s)
                if hasA:
                    alif_w(k)
                if has2:
                    v2_step(s)
                if hasA:
                    alif_sa(k)
                u_reset(s, has1, has2)
                if hasA:
                    alif_sth(k)
                    alif_ba(k)
                    alif_sub(k)
                if has1 or has2:
                    sig(s, has1, has2)
            return sz

        def emit_l2(i):
            s1 = S1[i]
            ax = axpool.tile([128, NC2 * NKB], F32, tag="AX", name="AX")
            AXB[i] = ax
            for m2 in range(NC2):
                pms = [pmpool.tile([128, 512], F32, tag="pm", name="pm")
                       for _ in range(NKB // 512)]
                for ph, wp in enumerate((w2hp, w2lp)):
                    for c in range(NC1):
                        for n in range(NKB // 512):
                            nc.tensor.matmul(
                                pms[n][:],
                                wp[c][m2][:],
                                s1[:, c * NKB + n * 512:c * NKB + (n + 1) * 512],
                                start=(ph == 0 and c == 0),
                                stop=(ph == 1 and c == NC1 - 1))
                for n in range(NKB // 512):
                    nc.scalar.activation(
                        ax[:, m2 * NKB + n * 512:m2 * NKB + (n + 1) * 512],
                        pms[n][:], ACTF.Identity, bias=b2t[:, m2:m2 + 1])

        def emit_l3(i):
            s1 = S1[i]
            sa = SA[i]
            x2 = x2pool.tile([128, NC1 * NKB], F32, tag="X2", name="X2")
            X2B[i] = x2
            for m in range(NC1):
                pms = [pmpool.tile([128, 512], F32, tag="pm", name="pm")
                       for _ in range(NKB // 512)]
                for ph, wp in enumerate((w3hp, w3lp)):
                    for c in range(NC1):
                        for n in range(NKB // 512):
                            nc.tensor.matmul(
                                pms[n][:],
                                wp[c][m][:],
                                s1[:, c * NKB + n * 512:c * NKB + (n + 1) * 512],
                                start=(ph == 0 and c == 0), stop=False)
                    for c2 in range(NC2):
                        for n in range(NKB // 512):
                            nc.tensor.matmul(
                                pms[n][:],
                                wp[NC1 + c2][m][:],
                                sa[:, c2 * NKB + n * 512:c2 * NKB + (n + 1) * 512],
                                start=False,
                                stop=(ph == 1 and c2 == NC2 - 1))
                for n in range(NKB // 512):
                    nc.scalar.activation(
                        x2[:, m * NKB + n * 512:m * NKB + (n + 1) * 512],
                        pms[n][:], ACTF.Identity, bias=b3t[:, m:m + 1])

        def emit_l4(i, sz):
            t0 = i * TB
            zbr = sz.rearrange("p (c t k b) -> p c t k b", c=2 * NC1, t=TB,
                               k=SIM)
            po = popool.tile([NOUT, NB], F32, tag="po", name="po")
            first = True
            for k in range(SIM):
                for c in range(NC1):
                    nc.tensor.matmul(
                        po[:], w4fp[k][c][:], zbr[:, NC1 + c, :, k, :],
                        start=first,
                        stop=(k == SIM - 1 and c == NC1 - 1))
                    first = False
            ot = opool.tile([NOUT, NB], F32, tag="OT", name="OT")
            nc.scalar.activation(ot[:], po[:], ACTF.Identity, bias=b4t[:, 0:1])
            nc.sync.dma_start(
                out=bass.AP(outT, t0 * BC, [[T * BC, NOUT], [1, NB]]),
                in_=ot[:])

        # software-pipelined emission with 2-round skew; L1 of the NEXT
        # block leads each round so PE has boundary work
        emit_inp_l1(0)
        for fn in HEAVY:
            fn()
        for r in range(NBLK + 2):
            if r + 1 < NBLK:
                emit_inp_l1(r + 1)
            zb = emit_chains(r)
            if 1 <= r < NBLK + 1:
                emit_l3(r - 1)
            if r < NBLK:
                emit_l2(r)
            if r >= 2:
                emit_l4(r - 2, zb)

    return nc


def _prep_host(inputs):
    inp = np.ascontiguousarray(inputs["inp"], dtype=np.float32)
    W1 = np.asarray(inputs["W1"], np.float32)
    W2 = np.asarray(inputs["W2"], np.float32)
    W3 = np.asarray(inputs["W3"], np.float32)
    W4 = np.asarray(inputs["W4"], np.float32)
    b1 = np.asarray(inputs["b1"], np.float32)
    b2 = np.asarray(inputs["b2"], np.float32)
    b3 = np.asarray(inputs["b3"], np.float32)
    b4 = np.asarray(inputs["b4"], np.float32)

    def split(W):
        # [K, M] -> [K//128, M//128, 128, 128] contiguous blocks (FWL needs
        # contiguous weight tiles)
        WT = W.T
        K, M = WT.shape
        Wh = WT.astype(ml_dtypes.bfloat16)
        Wl = (WT - Wh.astype(np.float32)).astype(np.float16)
        def blk(A):
            return np.ascontiguousarray(
                A.reshape(K // 128, 128, M // 128, 128).transpose(0, 2, 1, 3))
        return blk(Wh), blk(Wl)

    w2h, w2l = split(W2)
    w3h, w3l = split(W3)
    # W4cat: per sim step k the output integrator weight is beta^(SIM-1-k)*W4
    W4T = W4.T.astype(np.float64)                     # [NS2, NOUT]
    w4cat = np.stack([(BETA ** (SIM - 1 - k)) * W4T for k in range(SIM)])
    w4cat = w4cat.reshape(SIM, NC1, 128, NOUT)
    w4f_ = w4cat.astype(np.float16)
    csum = float(sum(BETA ** k for k in range(SIM)))
    shared = dict(
        w1t=np.ascontiguousarray(W1.T),
        w2h=w2h, w2l=w2l, w3h=w3h, w3l=w3l,
        w4f=np.ascontiguousarray(w4f_),
        b1m=np.ascontiguousarray(b1.reshape(NC1, 128)),
        b2m=np.ascontiguousarray(b2.reshape(NC2, 128)),
        b3m=np.ascontiguousarray(b3.reshape(NC1, 128)),
        b4c=np.ascontiguousarray((b4.astype(np.float64) * csum)
                                 .astype(np.float32).reshape(NOUT, 1)),
    )
    in_maps = []
    for c in range(NCORES):
        shard = inp[:, c * BC:(c + 1) * BC, :]                 # [T, BC, NIN]
        m = dict(shared)
        m["inpT"] = np.ascontiguousarray(shard.transpose(2, 0, 1))
        in_maps.append(m)
    return in_maps


def run(inputs, trace=False, **kw):
    if "nc" not in _CACHE:
        _CACHE["nc"] = build_nc()
    nc = _CACHE["nc"]
    in_maps = _prep_host(inputs)
    res = run_bass_kernel_spmd(nc, in_maps, core_ids=list(range(NCORES)),
                               trace=trace, **kw)
    outs = []
    for c in range(NCORES):
        outT = res.results[c]["outT"]                          # [NOUT, T, BC]
        outs.append(np.ascontiguousarray(outT.transpose(1, 2, 0)))
    full = np.concatenate(outs, axis=1)                        # [T, B, NOUT]
    return full, res


def kernel(**inputs):
    out, _ = run(inputs)
    return out


# revision 58
# speedup vs baseline: 1.3614x; 1.0041x over previous
"""Trainium2 Bass kernel for the AdaptiveFF spiking network.  (B1 state)

Sharding: data-parallel over batch, 8 NeuronCores, 32 batch elements per
core, weights replicated. No collectives needed.

Per-core kernel (all state feature-major: [feature_chunk=128, batch]):
  - X = inp @ W1.T is hoisted out of the sim loop (x_t is constant across
    the 4 sim steps) and batched per 8-step time block (fp32 matmuls).
  - The three recurrences (LIF v1, ALIF va/ba, LIF v2) are emitted as
    per-step interleaved DVE chains of three different time blocks
    (2-round software-pipeline skew; ALIF lags 8 steps), so the in-order
    engines always have independent work between the dependent ops of
    any one chain.
  - v1/v2 use a negated-state encoding u = s - w: the spike reset is a
    single fused STT (u = (w > 1) - w, exact fp32 parity with the
    reference) and the two chains' resets merge into one [128,256] op.
    v1 and v2 run at the same step index and their spike extractions
    merge into ONE saturated-sigmoid ACT op per step writing a shared
    s1|zb supertile (exact {0,1} outside ~1 ulp of threshold).
  - W2/W3 matmuls batch over (t, sim) with N=512 tiles and run as two
    passes: bf16 high + fp16 low residual, reproducing the fp32 product
    to ~1e-8 at 2x the fp32 rate. PSUM is evicted by ScalarE with the
    layer biases fused in.
  - The output integrator is folded into the W4 matmul by contracting
    over (sim_step, feature) against beta^(3-k)-scaled weight copies in
    a single fp16 pass (output-only path, no feedback).
Measured on trn2: ~1.274 ms HW exec, rel err ~0.0152 vs the fp32 numpy
reference (the fp32 chaos floor of this spiking net is ~0.013-0.014).
"""

import sys

for p in ("/opt/trn_rl_repo", "/root/.axon_site/_ro/trn_rl_repo"):
    if p not in sys.path:
        sys.path.append(p)

from contextlib import ExitStack

import numpy as np
import ml_dtypes

from concourse import mybir
import concourse.bass as bass
import concourse.tile as tile
from concourse.tile import TileContext
from concourse.bass_utils import run_bass_kernel_spmd

F32 = mybir.dt.float32
BF16 = mybir.dt.bfloat16
F16 = mybir.dt.float16
ALU = mybir.AluOpType
ACTF = mybir.ActivationFunctionType

T, B, NIN = 200, 256, 700
NS1, NA, NS2, NOUT = 512, 256, 512, 20
SIM = 4
BETA, THRESH, BETA_B, RHO = 0.9, 1.0, 0.95, 0.5
NCORES = 8
BC = B // NCORES          # 32 batch per core
TB = 8                    # time-block
NBLK = T // TB
NC1 = NS1 // 128          # 4 feature chunks for s1/x2
NC2 = NA // 128           # 2 chunks for sa/ax
NCI = (NIN + 127) // 128  # 6 input chunks (last ragged: 60)
NB = TB * BC              # 256 (t, b) cols per block
NKB = TB * SIM * BC       # 1024 (t, k, b) cols per block

_CACHE = {}


def _split_waits(nc, max_waits=1):
    """walrus in this container rejects >1 sem-wait per instruction; hoist
    extras onto preceding InstEventSemaphore instructions on the same
    engine (program order makes them happen-before)."""
    for f in nc.m.functions:
        for bb in f.blocks:
            dirty = False
            newl = []
            for ins in bb.instructions:
                si = ins.sync_info
                if si is not None and len(si.on_wait) > max_waits:
                    waits = list(si.on_wait)
                    for w in waits[:-max_waits]:
                        ev = mybir.InstEventSemaphore(
                            name=nc.get_next_instruction_name(), ins=[], outs=[])
                        ev.engine = ins.engine
                        ev.sync_info = mybir.SyncInfo(on_wait=[w], on_update=[])
                        nc.register_instruction(ev, overwrite=True)
                        newl.append(ev)
                    ins.sync_info = mybir.SyncInfo(
                        on_wait=waits[-max_waits:], on_update=list(si.on_update))
                    dirty = True
                newl.append(ins)
            if dirty:
                bb.instructions = newl


def _patch_tile_drain():
    if getattr(tile.TileContext, "_wait_split_patched", False):
        return
    orig = tile.TileContext._drain_and_barrier

    def patched(self, tick_clock, wait_clock):
        orig(self, tick_clock, wait_clock)
        _split_waits(self.nc)

    tile.TileContext._drain_and_barrier = patched
    tile.TileContext._wait_split_patched = True


def build_nc():
    _patch_tile_drain()
    nc = bass.Bass("TRN2", target_bir_lowering=False)

    dp = nc.declare_dram_parameter
    inpT = dp("inpT", [NIN, T, BC], F32, isOutput=False)
    w1t = dp("w1t", [NIN, NS1], F32, isOutput=False)
    w2h = dp("w2h", [NC1, NA // 128, 128, 128], BF16, isOutput=False)
    w2l = dp("w2l", [NC1, NA // 128, 128, 128], F16, isOutput=False)
    w3h = dp("w3h", [NC1 + NC2, NC1, 128, 128], BF16, isOutput=False)
    w3l = dp("w3l", [NC1 + NC2, NC1, 128, 128], F16, isOutput=False)
    w4f = dp("w4f", [SIM, NC1, 128, NOUT], F16, isOutput=False)
    b1m = dp("b1m", [NC1, 128], F32, isOutput=False)
    b2m = dp("b2m", [NC2, 128], F32, isOutput=False)
    b3m = dp("b3m", [NC1, 128], F32, isOutput=False)
    b4c = dp("b4c", [NOUT, 1], F32, isOutput=False)
    outT = dp("outT", [NOUT, T, BC], F32, isOutput=True)

    with TileContext(nc) as tc, ExitStack() as ctx:
        wpool = ctx.enter_context(tc.tile_pool(name="weights", bufs=1))
        spool = ctx.enter_context(tc.tile_pool(name="states", bufs=1))
        xpool = ctx.enter_context(tc.tile_pool(name="xbuf", bufs=3))
        szpool = ctx.enter_context(tc.tile_pool(name="szbuf", bufs=3))
        sapool = ctx.enter_context(tc.tile_pool(name="sabuf", bufs=3))
        axpool = ctx.enter_context(tc.tile_pool(name="axbuf", bufs=3))
        x2pool = ctx.enter_context(tc.tile_pool(name="x2buf", bufs=3))
        ipool = ctx.enter_context(tc.tile_pool(name="inp", bufs=3))
        opool = ctx.enter_context(tc.tile_pool(name="outt", bufs=2))
        pxpool = ctx.enter_context(tc.tile_pool(name="px", bufs=2, space="PSUM"))
        pmpool = ctx.enter_context(tc.tile_pool(name="pmid", bufs=5, space="PSUM"))
        popool = ctx.enter_context(tc.tile_pool(name="po", bufs=1, space="PSUM"))

        # ---- load weights ----
        w1 = []
        for c in range(NCI):
            kc = min(128, NIN - c * 128)
            wt = wpool.tile([kc, NS1], F32, tag=f"w1_{c}", name=f"w1_{c}")
            nc.sync.dma_start(out=wt[:], in_=w1t[c * 128:c * 128 + kc, :])
            w1.append(wt)

        def load_blocks(dram, nctot, nm, dt_, nm_name):
            # one DMA per K-chunk: [128, nm*128] tile whose m-th 128-col
            # slice is the contiguous [128,128] block (c, m)
            tiles = []
            for c in range(nctot):
                wt = wpool.tile([128, nm * 128], dt_, tag=f"{nm_name}_{c}",
                                name=f"{nm_name}_{c}")
                nc.sync.dma_start(
                    out=wt[:],
                    in_=bass.AP(dram, c * nm * 128 * 128,
                                [[128, 128], [128 * 128, nm], [1, 128]]))
                tiles.append([wt[:, m * 128:(m + 1) * 128] for m in range(nm)])
            return tiles

        w2hp, w3hp, w2lp, w3lp = [], [], [], []
        HEAVY = []
        HEAVY.append(lambda: w2hp.extend(load_blocks(w2h, NC1, NA // 128, BF16, "w2h")))
        HEAVY.append(lambda: w3hp.extend(load_blocks(w3h, NC1 + NC2, NC1, BF16, "w3h")))
        HEAVY.append(lambda: w2lp.extend(load_blocks(w2l, NC1, NA // 128, F16, "w2l")))
        HEAVY.append(lambda: w3lp.extend(load_blocks(w3l, NC1 + NC2, NC1, F16, "w3l")))
        w4fp = []

        def _load_w4():
            for k in range(SIM):
                rf = []
                for c in range(NC1):
                    wt = wpool.tile([128, NOUT], F16, tag=f"w4f_{k}_{c}",
                                    name=f"w4f_{k}_{c}")
                    nc.sync.dma_start(out=wt[:], in_=w4f[k, c, :, :])
                    rf.append(wt)
                w4fp.append(rf)
        HEAVY.append(_load_w4)
        b1t = wpool.tile([128, NC1], F32, tag="b1t", name="b1t")
        nc.sync.dma_start(out=b1t[:], in_=bass.AP(b1m, 0, [[1, 128], [128, NC1]]))
        b2t = wpool.tile([128, NC2], F32, tag="b2t", name="b2t")
        nc.sync.dma_start(out=b2t[:], in_=bass.AP(b2m, 0, [[1, 128], [128, NC2]]))
        b3t = wpool.tile([128, NC1], F32, tag="b3t", name="b3t")
        nc.sync.dma_start(out=b3t[:], in_=bass.AP(b3m, 0, [[1, 128], [128, NC1]]))
        b4t = wpool.tile([NOUT, 1], F32, tag="b4t", name="b4t")
        nc.sync.dma_start(out=b4t[:], in_=b4c[:, :])

        # ---- persistent states, layout [128, chunk*BC + b] ----
        SW = NC1 * BC
        AW = NC2 * BC
        wcat = [spool.tile([128, 2 * SW], F32, tag=f"wcat{j}", name=f"wcat{j}")
                for j in range(2)]
        ucat = spool.tile([128, 2 * SW], F32, tag="ucat", name="ucat")
        u1s = ucat[:, 0:SW]
        u2s = ucat[:, SW:2 * SW]
        va = spool.tile([128, AW], F32, tag="va", name="va")
        ba = spool.tile([128, AW], F32, tag="ba", name="ba")
        thr = spool.tile([128, AW], F32, tag="thr", name="thr")
        sth = spool.tile([128, AW], F32, tag="sth", name="sth")
        for st in (va, ba):
            nc.vector.memset(st[:], 0.0)
        nc.vector.memset(ucat[:], 0.0)

        # sigmoid-spike bias tile: s = sigmoid(SIGS*(v - THRESH)) saturates
        # to exact {0,1} outside ~1 ulp of the threshold
        SIGS = 1e8
        bsig = wpool.tile([128, 1], F32, tag="bsig", name="bsig")
        nc.vector.memset(bsig[:], -SIGS * THRESH)

        # per-block tiles carried between skewed emission rounds
        S1 = [None] * NBLK
        SA = [None] * NBLK
        AXB = [None] * NBLK
        X2B = [None] * NBLK
        XS = [None] * NBLK

        ITL = [None] * NBLK

        def emit_inp_dma(i):
            """issue block i's input DMAs (prefetched two rounds ahead so
            the transfers complete before the L1 matmuls need them)"""
            t0 = i * TB
            tiles = []
            for c in range(NCI):
                kc = min(128, NIN - c * 128)
                it = ipool.tile([kc, NB], F32, tag=f"inp_{c}", name=f"inp_{c}")
                nc.sync.dma_start(
                    out=it[:],
                    in_=bass.AP(inpT, c * 128 * T * BC + t0 * BC,
                                [[T * BC, kc], [1, NB]]))
                tiles.append(it)
            ITL[i] = tiles

        def emit_inp_l1(i):
            """L1 matmuls + X eviction for block i (inp DMA already issued)."""
            itiles = ITL[i]
            px = [pxpool.tile([128, 2 * NB], F32, tag="px", name="px")
                  for _ in range(2)]
            for mt in range(2):
                for m2 in range(2):
                    m = 2 * mt + m2
                    for c in range(NCI):
                        nc.tensor.matmul(
                            px[mt][:, m2 * NB:(m2 + 1) * NB],
                            w1[c][:, m * 128:(m + 1) * 128],
                            itiles[c][:],
                            start=(c == 0), stop=(c == NCI - 1))
            X = xpool.tile([128, NC1 * NB], F32, tag="X", name="X")
            XS[i] = X
            for m in range(NC1):
                nc.scalar.activation(
                    X[:, m * NB:(m + 1) * NB],
                    px[m // 2][:, (m % 2) * NB:(m % 2 + 1) * NB],
                    ACTF.Identity, bias=b1t[:, m:m + 1])

        def emit_chains(i):
            """Interleaved per-step emission of the three state chains:
            v1/s1 of block i, ALIF of block i-1 (lagged 8 steps so the
            L2(i-1) psum evictions land first), v2/s2/z of block i-2 (same
            step index as v1, so both spike extractions merge into one ACT
            op)."""
            NS = TB * SIM
            ALAG = 8
            s1 = saT = x2 = sz = None
            Xr = szr = axr = sar = x2r = None
            if 0 <= i < NBLK or 0 <= i - 2 < NBLK:
                # spike supertile: chunks 0..NC1-1 hold s1 of block i,
                # chunks NC1..2*NC1-1 hold s2 (zb) of block i-2
                sz = szpool.tile([128, 2 * NC1 * NKB], BF16, tag="SZ",
                                 name="SZ")
                szr = sz.rearrange("p (c t k b) -> p c t k b", c=2 * NC1,
                                   t=TB, k=SIM)
            if 0 <= i < NBLK:
                s1 = sz
                S1[i] = sz
                Xr = XS[i].rearrange("p (m t b) -> p m t b", m=NC1, t=TB)
            if 0 <= i - 1 < NBLK:
                ax = AXB[i - 1]
                saT = sapool.tile([128, NC2 * NKB], BF16, tag="SA", name="SA")
                SA[i - 1] = saT
                axr = ax.rearrange("p (c t k b) -> p c t k b", c=NC2, t=TB, k=SIM)
                sar = saT.rearrange("p (c t k b) -> p c t k b", c=NC2, t=TB, k=SIM)
                nc.scalar.activation(
                    thr[:], ba[:], ACTF.Identity, bias=THRESH, scale=RHO)
            if 0 <= i - 2 < NBLK:
                x2 = X2B[i - 2]
                x2r = x2.rearrange("p (c t k b) -> p c t k b", c=NC1, t=TB, k=SIM)

            def v1_step(s):
                tt, k = divmod(s, SIM)
                w = wcat[s % 2][:, 0:SW]
                nc.vector.scalar_tensor_tensor(
                    w, u1s, -BETA, Xr[:, :, tt, :], ALU.mult, ALU.add)

            def v2_step(s):
                tt, k = divmod(s, SIM)
                w = wcat[s % 2][:, SW:2 * SW]
                nc.vector.scalar_tensor_tensor(
                    w, u2s, -BETA, x2r[:, :, tt, k, :], ALU.mult, ALU.add)

            def u_reset(s, has1, has2):
                w = wcat[s % 2]
                if has1 and has2:
                    nc.vector.scalar_tensor_tensor(
                        ucat[:], w[:], THRESH, w[:], ALU.is_gt, ALU.subtract)
                elif has1:
                    nc.vector.scalar_tensor_tensor(
                        u1s, w[:, 0:SW], THRESH, w[:, 0:SW],
                        ALU.is_gt, ALU.subtract)
                elif has2:
                    nc.vector.scalar_tensor_tensor(
                        u2s, w[:, SW:2 * SW], THRESH, w[:, SW:2 * SW],
                        ALU.is_gt, ALU.subtract)

            def sig(s, has1, has2):
                """merged spike extraction for v1 (s1, chunks 0..NC1-1) and
                v2 (zb, chunks NC1..2*NC1-1) at the same step index"""
                tt, k = divmod(s, SIM)
                if has1 and has2:
                    nc.scalar.activation(
                        szr[:, :, tt, k, :],
                        wcat[s % 2].rearrange("p (c b) -> p c b", c=2 * NC1),
                        ACTF.Sigmoid, bias=bsig[:, 0:1], scale=SIGS)
                elif has1:
                    nc.scalar.activation(
                        szr[:, 0:NC1, tt, k, :],
                        wcat[s % 2][:, 0:SW].rearrange("p (c b) -> p c b",
                                                       c=NC1),
                        ACTF.Sigmoid, bias=bsig[:, 0:1], scale=SIGS)
                elif has2:
                    nc.scalar.activation(
                        szr[:, NC1:2 * NC1, tt, k, :],
                        wcat[s % 2][:, SW:2 * SW].rearrange("p (c b) -> p c b",
                                                            c=NC1),
                        ACTF.Sigmoid, bias=bsig[:, 0:1], scale=SIGS)

            def alif_w(s):
                tt, k = divmod(s, SIM)
                nc.vector.scalar_tensor_tensor(
                    va[:], va[:], BETA, axr[:, :, tt, k, :],
                    ALU.mult, ALU.add)

            def alif_sa(s):
                tt, k = divmod(s, SIM)
                nc.vector.tensor_tensor(
                    sar[:, :, tt, k, :], va[:], thr[:], ALU.is_gt)

            def alif_sth(s):
                tt, k = divmod(s, SIM)
                nc.vector.tensor_tensor(
                    sth[:], sar[:, :, tt, k, :], thr[:], ALU.mult)

            def alif_ba(s):
                tt, k = divmod(s, SIM)
                nc.vector.scalar_tensor_tensor(
                    ba[:], ba[:], BETA_B, sar[:, :, tt, k, :],
                    ALU.mult, ALU.add)
                # thr for the NEXT alif step, computed on ACT with a full
                # step of slack (thr = 1 + rho*ba)
                nc.scalar.activation(
                    thr[:], ba[:], ACTF.Identity, bias=THRESH, scale=RHO)

            def alif_sub(s):
                nc.vector.tensor_tensor(va[:], va[:], sth[:], ALU.subtract)

            # DVE order per step: v1w, aw, v2w, asa, u, asth, aba, asub --
            # every dependent pair (aw->asa, asa->asth, asth->asub,
            # asub->aw', u->v1w') is separated by at least one independent
            # op, so the in-order DVE never exposes a write->read SBUF
            # round-trip between back-to-back instructions.
            for s in range(NS + ALAG):
                has1 = s1 is not None and s < NS
                has2 = x2 is not None and s < NS
                hasA = saT is not None and ALAG <= s < NS + ALAG
                k = s - ALAG
                if has1:
                    v1_step(s)
                if hasA:
                    alif_w(k)
                if has2:
                    v2_step(s)
                if hasA:
                    alif_sa(k)
                u_reset(s, has1, has2)
                if hasA:
                    alif_sth(k)
                    alif_ba(k)
                    alif_sub(k)
                if has1 or has2:
                    sig(s, has1, has2)
            return sz

        def emit_l2(i):
            s1 = S1[i]
            ax = axpool.tile([128, NC2 * NKB], F32, tag="AX", name="AX")
            AXB[i] = ax
            for m2 in range(NC2):
                pms = [pmpool.tile([128, 512], F32, tag="pm", name="pm")
                       for _ in range(NKB // 512)]
                for ph, wp in enumerate((w2hp, w2lp)):
                    for c in range(NC1):
                        for n in range(NKB // 512):
                            nc.tensor.matmul(
                                pms[n][:],
                                wp[c][m2][:],
                                s1[:, c * NKB + n * 512:c * NKB + (n + 1) * 512],
                                start=(ph == 0 and c == 0),
                                stop=(ph == 1 and c == NC1 - 1))
                for n in range(NKB // 512):
                    nc.scalar.activation(
                        ax[:, m2 * NKB + n * 512:m2 * NKB + (n + 1) * 512],
                        pms[n][:], ACTF.Identity, bias=b2t[:, m2:m2 + 1])

        def emit_l3(i):
            s1 = S1[i]
            sa = SA[i]
            x2 = x2pool.tile([128, NC1 * NKB], F32, tag="X2", name="X2")
            X2B[i] = x2
            for m in range(NC1):
                pms = [pmpool.tile([128, 512], F32, tag="pm", name="pm")
                       for _ in range(NKB // 512)]
                for ph, wp in enumerate((w3hp, w3lp)):
                    for c in range(NC1):
                        for n in range(NKB // 512):
                            nc.tensor.matmul(
                                pms[n][:],
                                wp[c][m][:],
                                s1[:, c * NKB + n * 512:c * NKB + (n + 1) * 512],
                                start=(ph == 0 and c == 0), stop=False)
                    for c2 in range(NC2):
                        for n in range(NKB // 512):
                            nc.tensor.matmul(
                                pms[n][:],
                                wp[NC1 + c2][m][:],
                                sa[:, c2 * NKB + n * 512:c2 * NKB + (n + 1) * 512],
                                start=False,
                                stop=(ph == 1 and c2 == NC2 - 1))
                for n in range(NKB // 512):
                    nc.scalar.activation(
                        x2[:, m * NKB + n * 512:m * NKB + (n + 1) * 512],
                        pms[n][:], ACTF.Identity, bias=b3t[:, m:m + 1])

        def emit_l4(i, sz):
            t0 = i * TB
            zbr = sz.rearrange("p (c t k b) -> p c t k b", c=2 * NC1, t=TB,
                               k=SIM)
            po = popool.tile([NOUT, NB], F32, tag="po", name="po")
            first = True
            for k in range(SIM):
                for c in range(NC1):
                    nc.tensor.matmul(
                        po[:], w4fp[k][c][:], zbr[:, NC1 + c, :, k, :],
                        start=first,
                        stop=(k == SIM - 1 and c == NC1 - 1))
                    first = False
            ot = opool.tile([NOUT, NB], F32, tag="OT", name="OT")
            nc.scalar.activation(ot[:], po[:], ACTF.Identity, bias=b4t[:, 0:1])
            nc.sync.dma_start(
                out=bass.AP(outT, t0 * BC, [[T * BC, NOUT], [1, NB]]),
                in_=ot[:])

        # software-pipelined emission with 2-round skew; L1 of the NEXT
        # block leads each round so PE has boundary work; input DMAs are
        # prefetched two rounds ahead so transfers never gate the PE
        emit_inp_dma(0)
        emit_inp_dma(1)
        emit_inp_l1(0)
        for fn in HEAVY:
            fn()
        for r in range(NBLK + 2):
            if r + 2 < NBLK:
                emit_inp_dma(r + 2)
            if r + 1 < NBLK:
                emit_inp_l1(r + 1)
            zb = emit_chains(r)
            if 1 <= r < NBLK + 1:
                emit_l3(r - 1)
            if r < NBLK:
                emit_l2(r)
            if r >= 2:
                emit_l4(r - 2, zb)

    return nc


def _prep_host(inputs):
    inp = np.ascontiguousarray(inputs["inp"], dtype=np.float32)
    W1 = np.asarray(inputs["W1"], np.float32)
    W2 = np.asarray(inputs["W2"], np.float32)
    W3 = np.asarray(inputs["W3"], np.float32)
    W4 = np.asarray(inputs["W4"], np.float32)
    b1 = np.asarray(inputs["b1"], np.float32)
    b2 = np.asarray(inputs["b2"], np.float32)
    b3 = np.asarray(inputs["b3"], np.float32)
    b4 = np.asarray(inputs["b4"], np.float32)

    def split(W):
        # [K, M] -> [K//128, M//128, 128, 128] contiguous blocks (FWL needs
        # contiguous weight tiles)
        WT = W.T
        K, M = WT.shape
        Wh = WT.astype(ml_dtypes.bfloat16)
        Wl = (WT - Wh.astype(np.float32)).astype(np.float16)
        def blk(A):
            return np.ascontiguousarray(
                A.reshape(K // 128, 128, M // 128, 128).transpose(0, 2, 1, 3))
        return blk(Wh), blk(Wl)

    w2h, w2l = split(W2)
    w3h, w3l = split(W3)
    # W4cat: per sim step k the output integrator weight is beta^(SIM-1-k)*W4
    W4T = W4.T.astype(np.float64)                     # [NS2, NOUT]
    w4cat = np.stack([(BETA ** (SIM - 1 - k)) * W4T for k in range(SIM)])
    w4cat = w4cat.reshape(SIM, NC1, 128, NOUT)
    w4f_ = w4cat.astype(np.float16)
    csum = float(sum(BETA ** k for k in range(SIM)))
    shared = dict(
        w1t=np.ascontiguousarray(W1.T),
        w2h=w2h, w2l=w2l, w3h=w3h, w3l=w3l,
        w4f=np.ascontiguousarray(w4f_),
        b1m=np.ascontiguousarray(b1.reshape(NC1, 128)),
        b2m=np.ascontiguousarray(b2.reshape(NC2, 128)),
        b3m=np.ascontiguousarray(b3.reshape(NC1, 128)),
        b4c=np.ascontiguousarray((b4.astype(np.float64) * csum)
                                 .astype(np.float32).reshape(NOUT, 1)),
    )
    in_maps = []
    for c in range(NCORES):
        shard = inp[:, c * BC:(c + 1) * BC, :]                 # [T, BC, NIN]
        m = dict(shared)
        m["inpT"] = np.ascontiguousarray(shard.transpose(2, 0, 1))
        in_maps.append(m)
    return in_maps


def run(inputs, trace=False, **kw):
    if "nc" not in _CACHE:
        _CACHE["nc"] = build_nc()
    nc = _CACHE["nc"]
    in_maps = _prep_host(inputs)
    res = run_bass_kernel_spmd(nc, in_maps, core_ids=list(range(NCORES)),
                               trace=trace, **kw)
    outs = []
    for c in range(NCORES):
        outT = res.results[c]["outT"]                          # [NOUT, T, BC]
        outs.append(np.ascontiguousarray(outT.transpose(1, 2, 0)))
    full = np.concatenate(outs, axis=1)                        # [T, B, NOUT]
    return full, res


def kernel(**inputs):
    out, _ = run(inputs)
    return out


# revision 59
# speedup vs baseline: 1.3638x; 1.0018x over previous
"""Trainium2 Bass kernel for the AdaptiveFF spiking network.  (B1 state)

Sharding: data-parallel over batch, 8 NeuronCores, 32 batch elements per
core, weights replicated. No collectives needed.

Per-core kernel (all state feature-major: [feature_chunk=128, batch]):
  - X = inp @ W1.T is hoisted out of the sim loop (x_t is constant across
    the 4 sim steps) and batched per 8-step time block (fp32 matmuls).
  - The three recurrences (LIF v1, ALIF va/ba, LIF v2) are emitted as
    per-step interleaved DVE chains of three different time blocks
    (2-round software-pipeline skew; ALIF lags 8 steps), so the in-order
    engines always have independent work between the dependent ops of
    any one chain.
  - v1/v2 use a negated-state encoding u = s - w: the spike reset is a
    single fused STT (u = (w > 1) - w, exact fp32 parity with the
    reference) and the two chains' resets merge into one [128,256] op.
    v1 and v2 run at the same step index and their spike extractions
    merge into ONE saturated-sigmoid ACT op per step writing a shared
    s1|zb supertile (exact {0,1} outside ~1 ulp of threshold).
  - W2/W3 matmuls batch over (t, sim) with N=512 tiles and run as two
    passes: bf16 high + fp16 low residual, reproducing the fp32 product
    to ~1e-8 at 2x the fp32 rate. PSUM is evicted by ScalarE with the
    layer biases fused in.
  - The output integrator is folded into the W4 matmul by contracting
    over (sim_step, feature) against beta^(3-k)-scaled weight copies in
    a single fp16 pass (output-only path, no feedback).
Measured on trn2: ~1.274 ms HW exec, rel err ~0.0152 vs the fp32 numpy
reference (the fp32 chaos floor of this spiking net is ~0.013-0.014).
"""

import sys

for p in ("/opt/trn_rl_repo", "/root/.axon_site/_ro/trn_rl_repo"):
    if p not in sys.path:
        sys.path.append(p)

from contextlib import ExitStack

import numpy as np
import ml_dtypes

from concourse import mybir
import concourse.bass as bass
import concourse.tile as tile
from concourse.tile import TileContext
from concourse.bass_utils import run_bass_kernel_spmd

F32 = mybir.dt.float32
BF16 = mybir.dt.bfloat16
F16 = mybir.dt.float16
ALU = mybir.AluOpType
ACTF = mybir.ActivationFunctionType

T, B, NIN = 200, 256, 700
NS1, NA, NS2, NOUT = 512, 256, 512, 20
SIM = 4
BETA, THRESH, BETA_B, RHO = 0.9, 1.0, 0.95, 0.5
NCORES = 8
BC = B // NCORES          # 32 batch per core
TB = 8                    # time-block
NBLK = T // TB
NC1 = NS1 // 128          # 4 feature chunks for s1/x2
NC2 = NA // 128           # 2 chunks for sa/ax
NCI = (NIN + 127) // 128  # 6 input chunks (last ragged: 60)
NB = TB * BC              # 256 (t, b) cols per block
NKB = TB * SIM * BC       # 1024 (t, k, b) cols per block

_CACHE = {}


def _split_waits(nc, max_waits=1):
    """walrus in this container rejects >1 sem-wait per instruction; hoist
    extras onto preceding InstEventSemaphore instructions on the same
    engine (program order makes them happen-before)."""
    for f in nc.m.functions:
        for bb in f.blocks:
            dirty = False
            newl = []
            for ins in bb.instructions:
                si = ins.sync_info
                if si is not None and len(si.on_wait) > max_waits:
                    waits = list(si.on_wait)
                    for w in waits[:-max_waits]:
                        ev = mybir.InstEventSemaphore(
                            name=nc.get_next_instruction_name(), ins=[], outs=[])
                        ev.engine = ins.engine
                        ev.sync_info = mybir.SyncInfo(on_wait=[w], on_update=[])
                        nc.register_instruction(ev, overwrite=True)
                        newl.append(ev)
                    ins.sync_info = mybir.SyncInfo(
                        on_wait=waits[-max_waits:], on_update=list(si.on_update))
                    dirty = True
                newl.append(ins)
            if dirty:
                bb.instructions = newl


def _patch_tile_drain():
    if getattr(tile.TileContext, "_wait_split_patched", False):
        return
    orig = tile.TileContext._drain_and_barrier

    def patched(self, tick_clock, wait_clock):
        orig(self, tick_clock, wait_clock)
        _split_waits(self.nc)

    tile.TileContext._drain_and_barrier = patched
    tile.TileContext._wait_split_patched = True


def build_nc():
    _patch_tile_drain()
    nc = bass.Bass("TRN2", target_bir_lowering=False)

    dp = nc.declare_dram_parameter
    inpT = dp("inpT", [NIN, T, BC], F32, isOutput=False)
    w1t = dp("w1t", [NIN, NS1], F32, isOutput=False)
    w2h = dp("w2h", [NC1, NA // 128, 128, 128], BF16, isOutput=False)
    w2l = dp("w2l", [NC1, NA // 128, 128, 128], F16, isOutput=False)
    w3h = dp("w3h", [NC1 + NC2, NC1, 128, 128], BF16, isOutput=False)
    w3l = dp("w3l", [NC1 + NC2, NC1, 128, 128], F16, isOutput=False)
    w4f = dp("w4f", [SIM, NC1, 128, NOUT], F16, isOutput=False)
    b1m = dp("b1m", [NC1, 128], F32, isOutput=False)
    b2m = dp("b2m", [NC2, 128], F32, isOutput=False)
    b3m = dp("b3m", [NC1, 128], F32, isOutput=False)
    b4c = dp("b4c", [NOUT, 1], F32, isOutput=False)
    outT = dp("outT", [NOUT, T, BC], F32, isOutput=True)

    with TileContext(nc) as tc, ExitStack() as ctx:
        wpool = ctx.enter_context(tc.tile_pool(name="weights", bufs=1))
        spool = ctx.enter_context(tc.tile_pool(name="states", bufs=1))
        xpool = ctx.enter_context(tc.tile_pool(name="xbuf", bufs=3))
        szpool = ctx.enter_context(tc.tile_pool(name="szbuf", bufs=3))
        sapool = ctx.enter_context(tc.tile_pool(name="sabuf", bufs=3))
        axpool = ctx.enter_context(tc.tile_pool(name="axbuf", bufs=3))
        x2pool = ctx.enter_context(tc.tile_pool(name="x2buf", bufs=3))
        ipool = ctx.enter_context(tc.tile_pool(name="inp", bufs=3))
        opool = ctx.enter_context(tc.tile_pool(name="outt", bufs=2))
        pxpool = ctx.enter_context(tc.tile_pool(name="px", bufs=2, space="PSUM"))
        pmpool = ctx.enter_context(tc.tile_pool(name="pmid", bufs=5, space="PSUM"))
        popool = ctx.enter_context(tc.tile_pool(name="po", bufs=1, space="PSUM"))

        # ---- load weights ----
        w1 = []
        for c in range(NCI):
            kc = min(128, NIN - c * 128)
            wt = wpool.tile([kc, NS1], F32, tag=f"w1_{c}", name=f"w1_{c}")
            nc.sync.dma_start(out=wt[:], in_=w1t[c * 128:c * 128 + kc, :])
            w1.append(wt)

        def load_blocks(dram, nctot, nm, dt_, nm_name):
            # one DMA per K-chunk: [128, nm*128] tile whose m-th 128-col
            # slice is the contiguous [128,128] block (c, m)
            tiles = []
            for c in range(nctot):
                wt = wpool.tile([128, nm * 128], dt_, tag=f"{nm_name}_{c}",
                                name=f"{nm_name}_{c}")
                nc.sync.dma_start(
                    out=wt[:],
                    in_=bass.AP(dram, c * nm * 128 * 128,
                                [[128, 128], [128 * 128, nm], [1, 128]]))
                tiles.append([wt[:, m * 128:(m + 1) * 128] for m in range(nm)])
            return tiles

        w2hp, w3hp, w2lp, w3lp = [], [], [], []
        HEAVY = []
        HEAVY.append(lambda: w2hp.extend(load_blocks(w2h, NC1, NA // 128, BF16, "w2h")))
        HEAVY.append(lambda: w3hp.extend(load_blocks(w3h, NC1 + NC2, NC1, BF16, "w3h")))
        HEAVY.append(lambda: w2lp.extend(load_blocks(w2l, NC1, NA // 128, F16, "w2l")))
        HEAVY.append(lambda: w3lp.extend(load_blocks(w3l, NC1 + NC2, NC1, F16, "w3l")))
        w4fp = []

        def _load_w4():
            for k in range(SIM):
                rf = []
                for c in range(NC1):
                    wt = wpool.tile([128, NOUT], F16, tag=f"w4f_{k}_{c}",
                                    name=f"w4f_{k}_{c}")
                    nc.sync.dma_start(out=wt[:], in_=w4f[k, c, :, :])
                    rf.append(wt)
                w4fp.append(rf)
        HEAVY.append(_load_w4)
        b1t = wpool.tile([128, NC1], F32, tag="b1t", name="b1t")
        nc.sync.dma_start(out=b1t[:], in_=bass.AP(b1m, 0, [[1, 128], [128, NC1]]))
        b2t = wpool.tile([128, NC2], F32, tag="b2t", name="b2t")
        nc.sync.dma_start(out=b2t[:], in_=bass.AP(b2m, 0, [[1, 128], [128, NC2]]))
        b3t = wpool.tile([128, NC1], F32, tag="b3t", name="b3t")
        nc.sync.dma_start(out=b3t[:], in_=bass.AP(b3m, 0, [[1, 128], [128, NC1]]))
        b4t = wpool.tile([NOUT, 1], F32, tag="b4t", name="b4t")
        nc.sync.dma_start(out=b4t[:], in_=b4c[:, :])

        # ---- persistent states, layout [128, chunk*BC + b] ----
        SW = NC1 * BC
        AW = NC2 * BC
        wcat = [spool.tile([128, 2 * SW], F32, tag=f"wcat{j}", name=f"wcat{j}")
                for j in range(3)]
        ucat = spool.tile([128, 2 * SW], F32, tag="ucat", name="ucat")
        u1s = ucat[:, 0:SW]
        u2s = ucat[:, SW:2 * SW]
        va = spool.tile([128, AW], F32, tag="va", name="va")
        ba = spool.tile([128, AW], F32, tag="ba", name="ba")
        thr = spool.tile([128, AW], F32, tag="thr", name="thr")
        sth = spool.tile([128, AW], F32, tag="sth", name="sth")
        for st in (va, ba):
            nc.vector.memset(st[:], 0.0)
        nc.vector.memset(ucat[:], 0.0)

        # sigmoid-spike bias tile: s = sigmoid(SIGS*(v - THRESH)) saturates
        # to exact {0,1} outside ~1 ulp of the threshold
        SIGS = 1e8
        bsig = wpool.tile([128, 1], F32, tag="bsig", name="bsig")
        nc.vector.memset(bsig[:], -SIGS * THRESH)

        # per-block tiles carried between skewed emission rounds
        S1 = [None] * NBLK
        SA = [None] * NBLK
        AXB = [None] * NBLK
        X2B = [None] * NBLK
        XS = [None] * NBLK

        ITL = [None] * NBLK

        def emit_inp_dma(i):
            """issue block i's input DMAs (prefetched two rounds ahead so
            the transfers complete before the L1 matmuls need them)"""
            t0 = i * TB
            tiles = []
            for c in range(NCI):
                kc = min(128, NIN - c * 128)
                it = ipool.tile([kc, NB], F32, tag=f"inp_{c}", name=f"inp_{c}")
                nc.sync.dma_start(
                    out=it[:],
                    in_=bass.AP(inpT, c * 128 * T * BC + t0 * BC,
                                [[T * BC, kc], [1, NB]]))
                tiles.append(it)
            ITL[i] = tiles

        def emit_inp_l1(i):
            """L1 matmuls + X eviction for block i (inp DMA already issued)."""
            itiles = ITL[i]
            px = [pxpool.tile([128, 2 * NB], F32, tag="px", name="px")
                  for _ in range(2)]
            for mt in range(2):
                for m2 in range(2):
                    m = 2 * mt + m2
                    for c in range(NCI):
                        nc.tensor.matmul(
                            px[mt][:, m2 * NB:(m2 + 1) * NB],
                            w1[c][:, m * 128:(m + 1) * 128],
                            itiles[c][:],
                            start=(c == 0), stop=(c == NCI - 1))
            X = xpool.tile([128, NC1 * NB], F32, tag="X", name="X")
            XS[i] = X
            for m in range(NC1):
                nc.scalar.activation(
                    X[:, m * NB:(m + 1) * NB],
                    px[m // 2][:, (m % 2) * NB:(m % 2 + 1) * NB],
                    ACTF.Identity, bias=b1t[:, m:m + 1])

        def emit_chains(i):
            """Interleaved per-step emission of the three state chains:
            v1/s1 of block i, ALIF of block i-1 (lagged 8 steps so the
            L2(i-1) psum evictions land first), v2/s2/z of block i-2 (same
            step index as v1, so both spike extractions merge into one ACT
            op)."""
            NS = TB * SIM
            ALAG = 8
            s1 = saT = x2 = sz = None
            Xr = szr = axr = sar = x2r = None
            if 0 <= i < NBLK or 0 <= i - 2 < NBLK:
                # spike supertile: chunks 0..NC1-1 hold s1 of block i,
                # chunks NC1..2*NC1-1 hold s2 (zb) of block i-2
                sz = szpool.tile([128, 2 * NC1 * NKB], BF16, tag="SZ",
                                 name="SZ")
                szr = sz.rearrange("p (c t k b) -> p c t k b", c=2 * NC1,
                                   t=TB, k=SIM)
            if 0 <= i < NBLK:
                s1 = sz
                S1[i] = sz
                Xr = XS[i].rearrange("p (m t b) -> p m t b", m=NC1, t=TB)
            if 0 <= i - 1 < NBLK:
                ax = AXB[i - 1]
                saT = sapool.tile([128, NC2 * NKB], BF16, tag="SA", name="SA")
                SA[i - 1] = saT
                axr = ax.rearrange("p (c t k b) -> p c t k b", c=NC2, t=TB, k=SIM)
                sar = saT.rearrange("p (c t k b) -> p c t k b", c=NC2, t=TB, k=SIM)
                nc.scalar.activation(
                    thr[:], ba[:], ACTF.Identity, bias=THRESH, scale=RHO)
            if 0 <= i - 2 < NBLK:
                x2 = X2B[i - 2]
                x2r = x2.rearrange("p (c t k b) -> p c t k b", c=NC1, t=TB, k=SIM)

            def v1_step(s):
                tt, k = divmod(s, SIM)
                w = wcat[s % 3][:, 0:SW]
                nc.vector.scalar_tensor_tensor(
                    w, u1s, -BETA, Xr[:, :, tt, :], ALU.mult, ALU.add)

            def v2_step(s):
                tt, k = divmod(s, SIM)
                w = wcat[s % 3][:, SW:2 * SW]
                nc.vector.scalar_tensor_tensor(
                    w, u2s, -BETA, x2r[:, :, tt, k, :], ALU.mult, ALU.add)

            def u_reset(s, has1, has2):
                w = wcat[s % 3]
                if has1 and has2:
                    nc.vector.scalar_tensor_tensor(
                        ucat[:], w[:], THRESH, w[:], ALU.is_gt, ALU.subtract)
                elif has1:
                    nc.vector.scalar_tensor_tensor(
                        u1s, w[:, 0:SW], THRESH, w[:, 0:SW],
                        ALU.is_gt, ALU.subtract)
                elif has2:
                    nc.vector.scalar_tensor_tensor(
                        u2s, w[:, SW:2 * SW], THRESH, w[:, SW:2 * SW],
                        ALU.is_gt, ALU.subtract)

            def sig(s, has1, has2):
                """merged spike extraction for v1 (s1, chunks 0..NC1-1) and
                v2 (zb, chunks NC1..2*NC1-1) at the same step index"""
                tt, k = divmod(s, SIM)
                if has1 and has2:
                    nc.scalar.activation(
                        szr[:, :, tt, k, :],
                        wcat[s % 3].rearrange("p (c b) -> p c b", c=2 * NC1),
                        ACTF.Sigmoid, bias=bsig[:, 0:1], scale=SIGS)
                elif has1:
                    nc.scalar.activation(
                        szr[:, 0:NC1, tt, k, :],
                        wcat[s % 3][:, 0:SW].rearrange("p (c b) -> p c b",
                                                       c=NC1),
                        ACTF.Sigmoid, bias=bsig[:, 0:1], scale=SIGS)
                elif has2:
                    nc.scalar.activation(
                        szr[:, NC1:2 * NC1, tt, k, :],
                        wcat[s % 3][:, SW:2 * SW].rearrange("p (c b) -> p c b",
                                                            c=NC1),
                        ACTF.Sigmoid, bias=bsig[:, 0:1], scale=SIGS)

            def alif_w(s):
                tt, k = divmod(s, SIM)
                nc.vector.scalar_tensor_tensor(
                    va[:], va[:], BETA, axr[:, :, tt, k, :],
                    ALU.mult, ALU.add)

            def alif_sa(s):
                tt, k = divmod(s, SIM)
                nc.vector.tensor_tensor(
                    sar[:, :, tt, k, :], va[:], thr[:], ALU.is_gt)

            def alif_sth(s):
                tt, k = divmod(s, SIM)
                nc.vector.tensor_tensor(
                    sth[:], sar[:, :, tt, k, :], thr[:], ALU.mult)

            def alif_ba(s):
                tt, k = divmod(s, SIM)
                nc.vector.scalar_tensor_tensor(
                    ba[:], ba[:], BETA_B, sar[:, :, tt, k, :],
                    ALU.mult, ALU.add)
                # thr for the NEXT alif step, computed on ACT with a full
                # step of slack (thr = 1 + rho*ba)
                nc.scalar.activation(
                    thr[:], ba[:], ACTF.Identity, bias=THRESH, scale=RHO)

            def alif_sub(s):
                nc.vector.tensor_tensor(va[:], va[:], sth[:], ALU.subtract)

            # DVE order per step: v1w, aw, v2w, asa, u, asth, aba, asub --
            # every dependent pair (aw->asa, asa->asth, asth->asub,
            # asub->aw', u->v1w') is separated by at least one independent
            # op, so the in-order DVE never exposes a write->read SBUF
            # round-trip between back-to-back instructions.
            for s in range(NS + ALAG):
                has1 = s1 is not None and s < NS
                has2 = x2 is not None and s < NS
                hasA = saT is not None and ALAG <= s < NS + ALAG
                k = s - ALAG
                if has1:
                    v1_step(s)
                if hasA:
                    alif_w(k)
                if has2:
                    v2_step(s)
                if hasA:
                    alif_sa(k)
                u_reset(s, has1, has2)
                if hasA:
                    alif_sth(k)
                    alif_ba(k)
                    alif_sub(k)
                if has1 or has2:
                    sig(s, has1, has2)
            return sz

        def emit_l2(i):
            s1 = S1[i]
            ax = axpool.tile([128, NC2 * NKB], F32, tag="AX", name="AX")
            AXB[i] = ax
            for m2 in range(NC2):
                pms = [pmpool.tile([128, 512], F32, tag="pm", name="pm")
                       for _ in range(NKB // 512)]
                for ph, wp in enumerate((w2hp, w2lp)):
                    for c in range(NC1):
                        for n in range(NKB // 512):
                            nc.tensor.matmul(
                                pms[n][:],
                                wp[c][m2][:],
                                s1[:, c * NKB + n * 512:c * NKB + (n + 1) * 512],
                                start=(ph == 0 and c == 0),
                                stop=(ph == 1 and c == NC1 - 1))
                for n in range(NKB // 512):
                    nc.scalar.activation(
                        ax[:, m2 * NKB + n * 512:m2 * NKB + (n + 1) * 512],
                        pms[n][:], ACTF.Identity, bias=b2t[:, m2:m2 + 1])

        def emit_l3(i):
            s1 = S1[i]
            sa = SA[i]
            x2 = x2pool.tile([128, NC1 * NKB], F32, tag="X2", name="X2")
            X2B[i] = x2
            for m in range(NC1):
                pms = [pmpool.tile([128, 512], F32, tag="pm", name="pm")
                       for _ in range(NKB // 512)]
                for ph, wp in enumerate((w3hp, w3lp)):
                    for c in range(NC1):
                        for n in range(NKB // 512):
                            nc.tensor.matmul(
                                pms[n][:],
                                wp[c][m][:],
                                s1[:, c * NKB + n * 512:c * NKB + (n + 1) * 512],
                                start=(ph == 0 and c == 0), stop=False)
                    for c2 in range(NC2):
                        for n in range(NKB // 512):
                            nc.tensor.matmul(
                                pms[n][:],
                                wp[NC1 + c2][m][:],
                                sa[:, c2 * NKB + n * 512:c2 * NKB + (n + 1) * 512],
                                start=False,
                                stop=(ph == 1 and c2 == NC2 - 1))
                for n in range(NKB // 512):
                    nc.scalar.activation(
                        x2[:, m * NKB + n * 512:m * NKB + (n + 1) * 512],
                        pms[n][:], ACTF.Identity, bias=b3t[:, m:m + 1])

        def emit_l4(i, sz):
            t0 = i * TB
            zbr = sz.rearrange("p (c t k b) -> p c t k b", c=2 * NC1, t=TB,
                               k=SIM)
            po = popool.tile([NOUT, NB], F32, tag="po", name="po")
            first = True
            for k in range(SIM):
                for c in range(NC1):
                    nc.tensor.matmul(
                        po[:], w4fp[k][c][:], zbr[:, NC1 + c, :, k, :],
                        start=first,
                        stop=(k == SIM - 1 and c == NC1 - 1))
                    first = False
            ot = opool.tile([NOUT, NB], F32, tag="OT", name="OT")
            nc.scalar.activation(ot[:], po[:], ACTF.Identity, bias=b4t[:, 0:1])
            nc.sync.dma_start(
                out=bass.AP(outT, t0 * BC, [[T * BC, NOUT], [1, NB]]),
                in_=ot[:])

        # software-pipelined emission with 2-round skew; L1 of the NEXT
        # block leads each round so PE has boundary work; input DMAs are
        # prefetched two rounds ahead so transfers never gate the PE
        emit_inp_dma(0)
        emit_inp_dma(1)
        emit_inp_l1(0)
        for fn in HEAVY:
            fn()
        for r in range(NBLK + 2):
            if r + 2 < NBLK:
                emit_inp_dma(r + 2)
            if r + 1 < NBLK:
                emit_inp_l1(r + 1)
            zb = emit_chains(r)
            if 1 <= r < NBLK + 1:
                emit_l3(r - 1)
            if r < NBLK:
                emit_l2(r)
            if r >= 2:
                emit_l4(r - 2, zb)

    return nc


def _prep_host(inputs):
    inp = np.ascontiguousarray(inputs["inp"], dtype=np.float32)
    W1 = np.asarray(inputs["W1"], np.float32)
    W2 = np.asarray(inputs["W2"], np.float32)
    W3 = np.asarray(inputs["W3"], np.float32)
    W4 = np.asarray(inputs["W4"], np.float32)
    b1 = np.asarray(inputs["b1"], np.float32)
    b2 = np.asarray(inputs["b2"], np.float32)
    b3 = np.asarray(inputs["b3"], np.float32)
    b4 = np.asarray(inputs["b4"], np.float32)

    def split(W):
        # [K, M] -> [K//128, M//128, 128, 128] contiguous blocks (FWL needs
        # contiguous weight tiles)
        WT = W.T
        K, M = WT.shape
        Wh = WT.astype(ml_dtypes.bfloat16)
        Wl = (WT - Wh.astype(np.float32)).astype(np.float16)
        def blk(A):
            return np.ascontiguousarray(
                A.reshape(K // 128, 128, M // 128, 128).transpose(0, 2, 1, 3))
        return blk(Wh), blk(Wl)

    w2h, w2l = split(W2)
    w3h, w3l = split(W3)
    # W4cat: per sim step k the output integrator weight is beta^(SIM-1-k)*W4
    W4T = W4.T.astype(np.float64)                     # [NS2, NOUT]
    w4cat = np.stack([(BETA ** (SIM - 1 - k)) * W4T for k in range(SIM)])
    w4cat = w4cat.reshape(SIM, NC1, 128, NOUT)
    w4f_ = w4cat.astype(np.float16)
    csum = float(sum(BETA ** k for k in range(SIM)))
    shared = dict(
        w1t=np.ascontiguousarray(W1.T),
        w2h=w2h, w2l=w2l, w3h=w3h, w3l=w3l,
        w4f=np.ascontiguousarray(w4f_),
        b1m=np.ascontiguousarray(b1.reshape(NC1, 128)),
        b2m=np.ascontiguousarray(b2.reshape(NC2, 128)),
        b3m=np.ascontiguousarray(b3.reshape(NC1, 128)),
        b4c=np.ascontiguousarray((b4.astype(np.float64) * csum)
                                 .astype(np.float32).reshape(NOUT, 1)),
    )
    in_maps = []
    for c in range(NCORES):
        shard = inp[:, c * BC:(c + 1) * BC, :]                 # [T, BC, NIN]
        m = dict(shared)
        m["inpT"] = np.ascontiguousarray(shard.transpose(2, 0, 1))
        in_maps.append(m)
    return in_maps


def run(inputs, trace=False, **kw):
    if "nc" not in _CACHE:
        _CACHE["nc"] = build_nc()
    nc = _CACHE["nc"]
    in_maps = _prep_host(inputs)
    res = run_bass_kernel_spmd(nc, in_maps, core_ids=list(range(NCORES)),
                               trace=trace, **kw)
    outs = []
    for c in range(NCORES):
        outT = res.results[c]["outT"]                          # [NOUT, T, BC]
        outs.append(np.ascontiguousarray(outT.transpose(1, 2, 0)))
    full = np.concatenate(outs, axis=1)                        # [T, B, NOUT]
    return full, res


def kernel(**inputs):
    out, _ = run(inputs)
    return out
